# revision 19
# baseline (speedup 1.0000x reference)
"""MLA (multi-head latent attention) Bass kernel for TRN2, 8-core SPMD.

Sharding: DP over batch (2) x TP over heads (4 groups of 4 heads).
core c: batch b = c // 4, head-group g = c % 4 (heads 4g..4g+3).

Math (per core), v2 (d-major everywhere, no PE transposes):
  kv_aT   = Wkva^T x^T  (d-major [576, S]); ssq_kv via ones@sq matmuls
  kvnT    = kv_aT[:512] * rsqrt(mean sq)    (broadcast via PE)
  krT     = rope(kv_aT[512:]) d-major       (pair-swap via perm matmul)
  ssq_q   = ones @ (Wqa_slice^T x^T)^2 ; AllReduce -> rq
  qnT     = Wqn^T x^T (d-major per head) ; qrT = rope(Wqr^T x^T) packed 2-head
  knT     = Wkbk^T kvnT ; v = kvnT^T Wkbv
  e[k,q]  = exp(SCALE * (qT . kT)) * tril-window  (128-wide diag mask only)
  attnT   = (v^T e) / (1^T e)                      per head
  outT    = Wout_g^T @ attnT                       partial over heads, host sums
"""

import copy
import functools
import hashlib
from contextlib import ExitStack
import numpy as np
import ml_dtypes

import concourse.bass as bass
import concourse.mybir as mybir
import concourse.tile as tile
from concourse.masks import make_identity

F32 = mybir.dt.float32
F32R = mybir.dt.float32r
BF16 = mybir.dt.bfloat16
AF = mybir.ActivationFunctionType

B, S, D = 2, 1024, 2048
H, DN, DR, DV = 16, 128, 64, 128
RQ, RKV = 1536, 512
THETA = 10000.0
EPS = 1e-6
SCALE = float((DN + DR) ** -0.5)

NCORE = 8
TP = 4                  # head groups
HPG = H // TP           # 4 heads per core
NT = S // 128           # 8 token blocks
NTH = 2                 # 512-token halves
QTA = 512               # attention q-tile width
NQA = S // QTA          # 2 attention q tiles
KC = D // 128           # 16 contraction chunks over D
RC = RKV // 128         # 4 contraction chunks over RKV
WQA_SL = RQ // TP       # 384 per-core Wqa column slice (for ssq)
QCC = WQA_SL // 128     # 3 ssq chunks

SKIP, FREE, MIXED = 0, 1, 2

# (kb, qt) -> ('n', slot, coff) | ('w', slot), set by analyze_mask;
# consumed by build_program in the same process.
_MASK_SLOTS = None
_MASK_NS = _MASK_NW = 0


def build_program(block_cls, n_mixed, use_collective=True, wqa_cols=WQA_SL,
                  trn_type="TRN2", fix_waits=True, reps=1, level=6,
                  use_kv_ag=True, den_dve=True):
    """block_cls: dict[(kb, qt)] -> SKIP/FREE/MIXED; mixed blocks get a
    binmask window from the packed `masks` input per _MASK_SLOTS."""
    nc = bass.Bass(trn_type, num_devices=NCORE if use_collective else 1)
    mixed_slot = dict(_MASK_SLOTS) if _MASK_SLOTS is not None else {}
    ns, nw = _MASK_NS, _MASK_NW

    xT = nc.dram_tensor("xT", [D, S], BF16, kind="ExternalInput")
    wqa = nc.dram_tensor("wqa", [D, wqa_cols], BF16, kind="ExternalInput")
    wqn = nc.dram_tensor("wqn", [D, HPG * DN], BF16, kind="ExternalInput")
    wqr = nc.dram_tensor("wqr", [D, HPG * DR], BF16, kind="ExternalInput")
    wkva = nc.dram_tensor("wkva", [D, RKV + 2 * DR], BF16,
                      kind="ExternalInput")
    wkbk = nc.dram_tensor("wkbk", [RKV, HPG * DN], BF16, kind="ExternalInput")
    wkbv = nc.dram_tensor("wkbv", [RKV, HPG * DV], BF16, kind="ExternalInput")
    wout = nc.dram_tensor("wout", [HPG * DV, D], BF16, kind="ExternalInput")
    xkv = nc.dram_tensor("xkv", [D, S // TP], BF16, kind="ExternalInput")
    coskv = nc.dram_tensor("coskv", [128, S // TP], BF16,
                           kind="ExternalInput")
    sinkv = nc.dram_tensor("sinkv", [128, S // TP], BF16,
                           kind="ExternalInput")
    cosq = nc.dram_tensor("cosq", [128, S], BF16, kind="ExternalInput")
    sinq = nc.dram_tensor("sinq", [128, S], BF16, kind="ExternalInput")
    perm = nc.dram_tensor("perm", [128, 128], BF16, kind="ExternalInput")
    masks = nc.dram_tensor("masks", [128, max(nw * QTA + ns * 128, 128)],
                           BF16, kind="ExternalInput")
    outT = nc.dram_tensor("outT", [D, S], BF16, kind="ExternalOutput")

    with tile.TileContext(nc) as tc:
        with tc.tile_pool(name="p_const", bufs=1) as p_const:
            ones = p_const.tile([128, 1], BF16, tag="ones")
            nc.gpsimd.memset(ones[:], 1.0)
            epsc = p_const.tile([128, 1], F32, tag="epsc")
            nc.gpsimd.memset(epsc[:], EPS)
            ones_f = p_const.tile([1, 128], F32, tag="ones_f")
            nc.gpsimd.memset(ones_f[:], 1.0)
            ones_r = p_const.tile([1, 128], F32R, tag="ones_r")
            with nc.allow_low_precision(reason="f32r broadcast operand"):
                nc.scalar.copy(ones_r[:], ones_f[:])
            consts = (ones, epsc, ones_f, ones_r)
            for _rep in range(reps):
                with ExitStack() as ctx:
                    _emit(ctx, nc, tc, locals(), use_collective, wqa_cols,
                          block_cls, mixed_slot, ns, nw, level=level,
                          consts=consts, use_kv_ag=use_kv_ag,
                          den_dve=den_dve)
    if fix_waits:
        _fix_multiwait(nc)
    return nc


def _emit(ctx, nc, tc, t, use_collective, wqa_cols, block_cls, mixed_slot,
          ns, nw, level=6, consts=None, use_kv_ag=True, den_dve=True):
    # level: timing-probe cutoff. 0=DMA only, 1=+KV, 2=+L, 3=+Q, 4=+KB/QS,
    # 5=+attention, 6=full (default). Levels <6 produce garbage outputs.
    xT, wqa, wqn, wqr, wkva, wkbk, wkbv, wout = (
        t["xT"], t["wqa"], t["wqn"], t["wqr"], t["wkva"], t["wkbk"],
        t["wkbv"], t["wout"])
    cosq, sinq, perm, masks, outT = (t["cosq"], t["sinq"], t["perm"],
                                     t["masks"], t["outT"])
    xkv, coskv, sinkv = t["xkv"], t["coskv"], t["sinkv"]
    ones, epsc, ones_f, ones_r = consts

    # ---------------- persistent pools ----------------
    p_x = ctx.enter_context(tc.tile_pool(name="p_x", bufs=1))
    p_tab = ctx.enter_context(tc.tile_pool(name="p_tab", bufs=1))
    p_qn = ctx.enter_context(tc.tile_pool(name="p_qn", bufs=1))
    p_qr = ctx.enter_context(tc.tile_pool(name="p_qr", bufs=1))
    p_kn = ctx.enter_context(tc.tile_pool(name="p_kn", bufs=1))
    p_kr = ctx.enter_context(tc.tile_pool(name="p_kr", bufs=1))
    p_v = ctx.enter_context(tc.tile_pool(name="p_v", bufs=1))
    p_kvn = ctx.enter_context(tc.tile_pool(name="p_kvn", bufs=1))
    p_at = ctx.enter_context(tc.tile_pool(name="p_at", bufs=1))
    p_rq = ctx.enter_context(tc.tile_pool(name="p_rq", bufs=1))
    p_msk = ctx.enter_context(tc.tile_pool(name="p_msk", bufs=1))
    p_dram = ctx.enter_context(tc.tile_pool(name="p_dram", bufs=1,
                                            space="DRAM"))
    # weight pools: opened for the whole body; loads are emitted in phase
    # order so the DMA engine streams them in priority order (prefetch).
    qw = ctx.enter_context(tc.tile_pool(name="qw", bufs=1))
    bw = ctx.enter_context(tc.tile_pool(name="bw", bufs=1))
    ow = ctx.enter_context(tc.tile_pool(name="ow", bufs=1))
    if use_kv_ag:
        # keep weight pools resident all rep: enables cross-rep prefetch
        early = None
        kxw = ctx.enter_context(tc.tile_pool(name="kxw", bufs=1))
        lw = ctx.enter_context(tc.tile_pool(name="lw", bufs=1))
    else:
        early = ExitStack()  # kxw+lw close after phase L (frees SBUF)
        kxw = early.enter_context(tc.tile_pool(name="kxw", bufs=1))
        lw = early.enter_context(tc.tile_pool(name="lw", bufs=1))

    # --- input DMAs, all on the SP queue in consumption order so each
    # phase's weights land before the phase starts: x+wkva (KV), wqa (L),
    # tables+perm (KV rope), wqn/wqr (Q), wkbk/wkbv (KB), masks (A),
    # wout (O).
    TQ = S // TP
    xt_sb, wkva_sb, xkv_sb = [], [], []
    if use_kv_ag:
        xkv_all = kxw.tile([128, KC * TQ], BF16, tag="xkv")
        nc.sync.dma_start(
            xkv_all[:].rearrange("p (c s) -> p c s", c=KC),
            xkv[:, :].rearrange("(c p) s -> p c s", p=128))
        xkv_sb = [xkv_all[:, kc * TQ:(kc + 1) * TQ] for kc in range(KC)]
        coskv_sb = p_tab.tile([128, TQ], BF16, tag="coskv")
        sinkv_sb = p_tab.tile([128, TQ], BF16, tag="sinkv")
        permkv_sb = p_tab.tile([128, 128], BF16, tag="permkv")
        nc.sync.dma_start(coskv_sb[:], coskv[:, :])
        nc.sync.dma_start(sinkv_sb[:], sinkv[:, :])
        nc.sync.dma_start(permkv_sb[:], perm[:, :])
    for i in range(4):
        wg = kxw.tile([128, 4 * (RKV + 2 * DR)], BF16, tag=f"wkva{i}")
        nc.sync.dma_start(
            wg[:].rearrange("p (c s) -> p c s", c=4),
            wkva[i * 512:(i + 1) * 512, :]
            .rearrange("(c p) s -> p c s", p=128))
        wkva_sb += [wg[:, j * (RKV + 2 * DR):(j + 1) * (RKV + 2 * DR)]
                    for j in range(4)]
    for i in range(4):
        xg = p_x.tile([128, 4 * S], BF16, tag=f"x{i}")
        nc.sync.dma_start(
            xg[:].rearrange("p (c s) -> p c s", c=4),
            xT[i * 512:(i + 1) * 512, :]
            .rearrange("(c p) s -> p c s", p=128))
        xt_sb += [xg[:, j * S:(j + 1) * S] for j in range(4)]

    cos_sb = p_tab.tile([128, S], BF16, tag="cos")
    sin_sb = p_tab.tile([128, S], BF16, tag="sin")
    perm_sb = p_tab.tile([128, 128], BF16, tag="perm")
    nc.sync.dma_start(cos_sb[:], cosq[:, :])
    nc.sync.dma_start(sin_sb[:], sinq[:, :])
    nc.sync.dma_start(perm_sb[:], perm[:, :])

    wqa_all = lw.tile([128, KC * wqa_cols], BF16, tag="wqa")
    wqa_sb = [wqa_all[:, kc * wqa_cols:(kc + 1) * wqa_cols]
              for kc in range(KC)]
    for i in range(4):
        nc.sync.dma_start(
            wqa_all[:, i * 4 * wqa_cols:(i + 1) * 4 * wqa_cols]
            .rearrange("p (c s) -> p c s", c=4),
            wqa[i * 512:(i + 1) * 512, :].rearrange("(c p) s -> p c s",
                                                    p=128))
    wqn_all = qw.tile([128, KC * HPG * DN], BF16, tag="wqn")
    wqr_all = qw.tile([128, KC * HPG * DR], BF16, tag="wqr")
    wqn_sb = [wqn_all[:, kc * HPG * DN:(kc + 1) * HPG * DN]
              for kc in range(KC)]
    wqr_sb = [wqr_all[:, kc * HPG * DR:(kc + 1) * HPG * DR]
              for kc in range(KC)]
    for i in range(4):
        nc.sync.dma_start(
            wqn_all[:, i * 4 * HPG * DN:(i + 1) * 4 * HPG * DN]
            .rearrange("p (c s) -> p c s", c=4),
            wqn[i * 512:(i + 1) * 512, :].rearrange("(c p) s -> p c s",
                                                    p=128))
        nc.sync.dma_start(
            wqr_all[:, i * 4 * HPG * DR:(i + 1) * 4 * HPG * DR]
            .rearrange("p (c s) -> p c s", c=4),
            wqr[i * 512:(i + 1) * 512, :].rearrange("(c p) s -> p c s",
                                                    p=128))
    wkbk_sb, wkbv_sb = [], []
    for rc in range(RC):
        wc = bw.tile([128, HPG * DN], BF16, tag=f"wkbk{rc}")
        nc.sync.dma_start(wc[:], wkbk[rc * 128:(rc + 1) * 128, :])
        wkbk_sb.append(wc)
        wc2 = bw.tile([128, HPG * DV], BF16, tag=f"wkbv{rc}")
        nc.sync.dma_start(wc2[:], wkbv[rc * 128:(rc + 1) * 128, :])
        wkbv_sb.append(wc2)
    mw = max(nw * QTA + ns * 128, 128)
    msk_sb = p_msk.tile([128, mw], BF16, tag="msk")
    nc.sync.dma_start(msk_sb[:], masks[:, :mw])
    wout_sb = []
    for hc in range(HPG):
        wc = ow.tile([128, D], BF16, tag=f"wo{hc}")
        nc.sync.dma_start(wc[:], wout[hc * 128:(hc + 1) * 128, :])
        wout_sb.append(wc)

    def _noop_out():
        # timing probes: still write outT so the program has its output
        for mb in range(D // 128):
            nc.sync.dma_start(outT[mb * 128:(mb + 1) * 128, 0:512],
                              xt_sb[0][:, 0:512])
        return

    # ------- phase KV: kv_aT d-major + rms + rope (no transposes) -------
    if level < 1:
        _noop_out()
        if early is not None:
            early.close()
        return
    kvnT = [p_kvn.tile([128, S], BF16, tag=f"kvn{rc}", name=f"kvn{rc}")
            for rc in range(RC)]
    krT = p_kr.tile([128, S], BF16, tag="krT")
    if use_kv_ag:
        # each core: its S/TP-token slice, fully normalized + roped, then
        # one AllGather of the packed [128, (RC+1)*TQ] result.
        kvag_in = p_dram.tile([128 * (RC + 1) * TQ], BF16, name="kvag_in")
        kvag_out = p_dram.tile([TP * 128 * (RC + 1) * TQ], BF16,
                               name="kvag_out")
        with tc.tile_pool(name="kp", bufs=2, space="PSUM") as kp, \
             tc.tile_pool(name="ksp", bufs=1, space="PSUM") as ksp, \
             tc.tile_pool(name="ks", bufs=2) as ks:
            kq = ks.tile([128, (RC + 1) * TQ], BF16, tag="kq", bufs=1)
            sqs = []
            for rc in range(RC):
                ps = kp.tile([128, TQ], F32, tag="kva")
                for kc in range(KC):
                    nc.tensor.matmul(ps[:],
                                     wkva_sb[kc][:, rc * 128:(rc + 1) * 128],
                                     xkv_sb[kc][:],
                                     start=(kc == 0), stop=(kc == KC - 1))
                nc.scalar.copy(kq[:, rc * TQ:(rc + 1) * TQ], ps[:])
                sq = ks.tile([128, TQ], BF16, tag=f"sq{rc}", bufs=1)
                nc.scalar.activation(sq[:], ps[:], AF.Square)
                sqs.append(sq)
            pr = kp.tile([128, TQ], F32, tag="kva")
            for kc in range(KC):
                nc.tensor.matmul(pr[:], wkva_sb[kc][:, RKV:], xkv_sb[kc][:],
                                 start=(kc == 0), stop=(kc == KC - 1))
            krq = ks.tile([128, TQ], BF16, tag="krq", bufs=1)
            nc.scalar.copy(krq[:], pr[:])
            sp_ = ksp.tile([1, TQ], F32, tag="ssq")
            for rc in range(RC):
                nc.tensor.matmul(sp_[:], ones[:, 0:1], sqs[rc][:],
                                 start=(rc == 0), stop=(rc == RC - 1))
            ssq_sb = ks.tile([1, TQ], F32, tag="ssq_sb", bufs=1)
            nc.vector.tensor_copy(ssq_sb[:], sp_[:])
            rk = ks.tile([1, TQ], F32, tag="rk", bufs=1)
            nc.scalar.activation(rk[:], ssq_sb[:], AF.Sqrt, scale=1.0 / RKV,
                                 bias=epsc[0:1, 0:1])
            sk = ks.tile([1, TQ], F32R, tag="sk", bufs=1)
            with nc.allow_low_precision(reason="f32r broadcast operand"):
                nc.vector.reciprocal(sk[:], rk[:])
            pb = kp.tile([128, TQ], F32, tag="kva")
            nc.tensor.matmul(pb[:], ones_r[:], sk[:], start=True, stop=True)
            skq = ks.tile([128, TQ], F32, tag="skq", bufs=1)
            nc.vector.tensor_copy(skq[:], pb[:])
            for rc in range(RC):
                nc.vector.tensor_mul(kq[:, rc * TQ:(rc + 1) * TQ],
                                     kq[:, rc * TQ:(rc + 1) * TQ], skq[:])
            # rope on the quarter
            m1 = ks.tile([128, TQ], BF16, tag="m1", bufs=1)
            nc.vector.tensor_mul(m1[:], krq[:], coskv_sb[:])
            prs = kp.tile([128, TQ], F32, tag="kva")
            nc.tensor.matmul(prs[:], permkv_sb[:], krq[:],
                             start=True, stop=True)
            t2 = ks.tile([128, TQ], F32, tag="t2", bufs=1)
            nc.vector.tensor_mul(t2[:], prs[:], sinkv_sb[:])
            nc.vector.tensor_add(kq[:, RC * TQ:(RC + 1) * TQ], m1[:], t2[:])
            nc.scalar.dma_start(
                kvag_in[:].rearrange("(p f) -> p f", p=128), kq[:])
        if use_collective:
            nc.gpsimd.collective_compute(
                "AllGather", mybir.AluOpType.bypass,
                replica_groups=[[0, 1, 2, 3], [4, 5, 6, 7]],
                ins=[kvag_in.opt()], outs=[kvag_out.opt()])

        def _kv_readback():
            # emitted after phase Q: the AllGather is long done by then, so
            # these triggers never stall the ACT stream.
            if not use_collective:
                for g in range(TP):
                    nc.scalar.dma_start(
                        kvag_out[g * 128 * (RC + 1) * TQ:
                                 (g + 1) * 128 * (RC + 1) * TQ],
                        kvag_in[:])
            for g in range(TP):
                view = kvag_out[g * 128 * (RC + 1) * TQ:
                                (g + 1) * 128 * (RC + 1) * TQ] \
                    .rearrange("(p f) -> p f", p=128)
                for rc in range(RC):
                    nc.scalar.dma_start(kvnT[rc][:, g * TQ:(g + 1) * TQ],
                                        view[:, rc * TQ:(rc + 1) * TQ])
                nc.scalar.dma_start(krT[:, g * TQ:(g + 1) * TQ],
                                    view[:, RC * TQ:(RC + 1) * TQ])
    else:
        _kv_readback = None
        krraw = p_kr.tile([128, S], BF16, tag="krraw")
        sk_bc = p_kr.tile([128, S], F32, tag="skbc")
        with tc.tile_pool(name="kp", bufs=2, space="PSUM") as kp, \
             tc.tile_pool(name="kp2", bufs=2, space="PSUM") as kp2, \
             tc.tile_pool(name="ksp", bufs=2, space="PSUM") as ksp, \
             tc.tile_pool(name="ks", bufs=2) as ks:
            ssq_ps = []
            for th in range(NTH):
                sqs = []
                for rc in range(RC):
                    ps = kp.tile([128, 512], F32, tag="kva")
                    for kc in range(KC):
                        nc.tensor.matmul(ps[:],
                                         wkva_sb[kc][:, rc * 128:(rc + 1) * 128],
                                         xt_sb[kc][:, th * 512:(th + 1) * 512],
                                         start=(kc == 0), stop=(kc == KC - 1))
                    nc.scalar.copy(kvnT[rc][:, th * 512:(th + 1) * 512], ps[:])
                    sq = ks.tile([128, 512], BF16, tag=f"sq{rc}", bufs=2)
                    nc.scalar.activation(sq[:], ps[:], AF.Square)
                    sqs.append(sq)
                pr = kp2.tile([128, 512], F32, tag="kvr")
                for kc in range(KC):
                    nc.tensor.matmul(pr[:], wkva_sb[kc][:, RKV:],
                                     xt_sb[kc][:, th * 512:(th + 1) * 512],
                                     start=(kc == 0), stop=(kc == KC - 1))
                nc.scalar.copy(krraw[:, th * 512:(th + 1) * 512], pr[:])
                sp_ = ksp.tile([1, 512], F32, tag="ssq")
                for rc in range(RC):
                    nc.tensor.matmul(sp_[:], ones[:, 0:1], sqs[rc][:],
                                     start=(rc == 0), stop=(rc == RC - 1))
                ssq_ps.append(sp_)
            # rsqrt chain on [1, S]
            ssq_sb = ks.tile([1, S], F32, tag="ssq_sb", bufs=1)
            for th in range(NTH):
                nc.vector.tensor_copy(ssq_sb[:, th * 512:(th + 1) * 512],
                                      ssq_ps[th][:])
            rk = ks.tile([1, S], F32, tag="rk", bufs=1)
            nc.scalar.activation(rk[:], ssq_sb[:], AF.Sqrt, scale=1.0 / RKV,
                                 bias=epsc[0:1, 0:1])
            sk = ks.tile([1, S], F32R, tag="sk", bufs=1)
            with nc.allow_low_precision(reason="f32r broadcast operand"):
                nc.vector.reciprocal(sk[:], rk[:])
            # broadcast to [128, S] via PE, then scale kvnT in place
            for th in range(NTH):
                pb = kp.tile([128, 512], F32, tag="kva")
                nc.tensor.matmul(pb[:], ones_r[:],
                                 sk[:, th * 512:(th + 1) * 512],
                                 start=True, stop=True)
                nc.vector.tensor_copy(sk_bc[:, th * 512:(th + 1) * 512],
                                      pb[:])
            for rc in range(RC):
                nc.vector.tensor_mul(kvnT[rc][:], kvnT[rc][:], sk_bc[:])
            # rope on krraw (d-major): krT = krraw*cos + (perm@krraw)*sin'
            m1 = ks.tile([128, S], BF16, tag="m1", bufs=1)
            nc.vector.tensor_mul(m1[:], krraw[:], cos_sb[:])
            for th in range(NTH):
                prs = kp2.tile([128, 512], F32, tag="prs", bufs=1)
                nc.tensor.matmul(prs[:], perm_sb[:],
                                 krraw[:, th * 512:(th + 1) * 512],
                                 start=True, stop=True)
                t2 = ks.tile([128, 512], F32, tag="t2")
                nc.vector.tensor_mul(t2[:], prs[:],
                                     sin_sb[:, th * 512:(th + 1) * 512])
                nc.vector.tensor_add(krT[:, th * 512:(th + 1) * 512],
                                     m1[:, th * 512:(th + 1) * 512], t2[:])

    # ---------------- phase L: q ssq (partial) + AllReduce ----------------
    if level < 2:
        _noop_out()
        if early is not None:
            early.close()
        return
    ssq_in = p_dram.tile([S], F32)
    ssq_out = p_dram.tile([S], F32)
    with tc.tile_pool(name="lp", bufs=2, space="PSUM") as lp, \
         tc.tile_pool(name="lsp", bufs=2, space="PSUM") as lsp, \
         tc.tile_pool(name="ls", bufs=2) as ls:
        qssq_sb = ls.tile([1, S], F32, tag="qssq", bufs=1)
        for th in range(NTH):
            sqs = []
            for cc in range(QCC):
                ps = lp.tile([128, 512], F32, tag="qa")
                for kc in range(KC):
                    nc.tensor.matmul(ps[:],
                                     wqa_sb[kc][:, cc * 128:(cc + 1) * 128],
                                     xt_sb[kc][:, th * 512:(th + 1) * 512],
                                     start=(kc == 0), stop=(kc == KC - 1))
                sq = ls.tile([128, 512], BF16, tag=f"qsq{cc}", bufs=2)
                nc.scalar.activation(sq[:], ps[:], AF.Square)
                sqs.append(sq)
            sp_ = lsp.tile([1, 512], F32, tag="qssqp")
            for cc in range(QCC):
                nc.tensor.matmul(sp_[:], ones[:, 0:1], sqs[cc][:],
                                 start=(cc == 0), stop=(cc == QCC - 1))
            nc.vector.tensor_copy(qssq_sb[:, th * 512:(th + 1) * 512],
                                  sp_[:])
        nc.scalar.dma_start(ssq_in[:].rearrange("(one s) -> one s",
                                                one=1), qssq_sb[:])
    if early is not None:
        early.close()
    if use_collective:
        nc.gpsimd.collective_compute(
            "AllReduce", mybir.AluOpType.add,
            replica_groups=[[0, 1, 2, 3], [4, 5, 6, 7]],
            ins=[ssq_in.opt()], outs=[ssq_out.opt()])
    else:
        nc.sync.dma_start(ssq_out[:], ssq_in[:])

    # ------- phase Q: qr (packed 2-head, d-major rope) + q_nope -------
    if level < 3:
        _noop_out()
        return
    qnT = [p_qn.tile([128, S], BF16, tag=f"qn{h}", name=f"qn{h}")
           for h in range(HPG)]
    qrP = [p_qr.tile([128, S], BF16, tag=f"qr{p}", name=f"qr{p}")
           for p in range(HPG // 2)]
    with tc.tile_pool(name="qp", bufs=2, space="PSUM") as qp, \
         tc.tile_pool(name="qpt", bufs=2, space="PSUM") as qpt, \
         tc.tile_pool(name="qs", bufs=2) as qs:
        qrraw = [qs.tile([128, S], BF16, tag=f"qrr{p}", bufs=1,
                         name=f"qrr{p}")
                 for p in range(HPG // 2)]
        for p in range(HPG // 2):
            for th in range(NTH):
                ps = qp.tile([128, 512], F32, tag="qr")
                for kc in range(KC):
                    nc.tensor.matmul(ps[:],
                                     wqr_sb[kc][:, p * 128:(p + 1) * 128],
                                     xt_sb[kc][:, th * 512:(th + 1) * 512],
                                     start=(kc == 0), stop=(kc == KC - 1))
                nc.scalar.copy(qrraw[p][:, th * 512:(th + 1) * 512], ps[:])
        for h in range(HPG):
            for th in range(NTH):
                ps = qp.tile([128, 512], F32, tag="qn")
                for kc in range(KC):
                    nc.tensor.matmul(
                        ps[:], wqn_sb[kc][:, h * DN:(h + 1) * DN],
                        xt_sb[kc][:, th * 512:(th + 1) * 512],
                        start=(kc == 0), stop=(kc == KC - 1))
                nc.scalar.copy(qnT[h][:, th * 512:(th + 1) * 512], ps[:])
        # d-major rope on packed qr pairs (DVE + perm matmuls)
        for p in range(HPG // 2):
            m1 = qs.tile([128, S], BF16, tag="qm1", bufs=2)
            nc.vector.tensor_mul(m1[:], qrraw[p][:], cos_sb[:])
            for th in range(NTH):
                prs = qpt.tile([128, 512], F32, tag="qprs")
                nc.tensor.matmul(prs[:], perm_sb[:],
                                 qrraw[p][:, th * 512:(th + 1) * 512],
                                 start=True, stop=True)
                t2 = qs.tile([128, 512], F32, tag="qt2", bufs=4)
                nc.vector.tensor_mul(t2[:], prs[:],
                                     sin_sb[:, th * 512:(th + 1) * 512])
                nc.vector.tensor_add(qrP[p][:, th * 512:(th + 1) * 512],
                                     m1[:, th * 512:(th + 1) * 512], t2[:])

    # rq recip chain + kv readback, emitted after phase Q so the r1 /
    # readback DMA triggers never stall the ACT stream on the collectives.
    rqs = ctx.enter_context(tc.tile_pool(name="rqs", bufs=1))
    r1 = rqs.tile([1, S], F32, tag="r1")
    nc.scalar.dma_start(r1[:],
                        ssq_out[:].rearrange("(one s) -> one s", one=1))
    r2 = rqs.tile([1, S], F32, tag="r2")
    nc.scalar.activation(r2[:], r1[:], AF.Sqrt, scale=1.0 / RQ,
                         bias=epsc[0:1, 0:1])
    r3 = rqs.tile([1, S], F32R, tag="r3")
    with nc.allow_low_precision(reason="f32r broadcast operand"):
        nc.vector.reciprocal(r3[:], r2[:])
    if _kv_readback is not None:
        _kv_readback()

    # ------- phase QS: broadcast 1/rms_q to [128, S] via PE -------
    if level < 4:
        _noop_out()
        return
    rq_bc = p_rq.tile([128, S], F32, tag="rq")
    with tc.tile_pool(name="rqp", bufs=2, space="PSUM") as rqp:
        for th in range(NTH):
            pb = rqp.tile([128, 512], F32, tag="pb")
            nc.tensor.matmul(pb[:], ones_r[:],
                             r3[:, th * 512:(th + 1) * 512],
                             start=True, stop=True)
            nc.scalar.copy(rq_bc[:, th * 512:(th + 1) * 512], pb[:])

    # ---------------- phase KB: k_nope (d-major) + v (token-major) ----------
    knT = [p_kn.tile([128, S], BF16, tag=f"kn{h}", name=f"kn{h}")
           for h in range(HPG)]
    v_sb = [p_v.tile([128, HPG * DV], BF16, tag=f"v{tb}", name=f"v{tb}")
            for tb in range(NT)]
    with tc.tile_pool(name="bp", bufs=2, space="PSUM") as bp:
        for h in range(HPG):
            for th in range(NTH):
                ps = bp.tile([128, 512], F32, tag="kn")
                for rc in range(RC):
                    nc.tensor.matmul(
                        ps[:], wkbk_sb[rc][:, h * DN:(h + 1) * DN],
                        kvnT[rc][:, th * 512:(th + 1) * 512],
                        start=(rc == 0), stop=(rc == RC - 1))
                nc.scalar.copy(knT[h][:, th * 512:(th + 1) * 512], ps[:])
        for tb in range(NT):
            ps = bp.tile([128, HPG * DV], F32, tag="v")
            for rc in range(RC):
                nc.tensor.matmul(ps[:], kvnT[rc][:, tb * 128:(tb + 1) * 128],
                                 wkbv_sb[rc][:], start=(rc == 0),
                                 stop=(rc == RC - 1))
            nc.scalar.copy(v_sb[tb][:], ps[:])

    # --- phase A: attention (transposed flash), software-pipelined, qt-major
    # interleaved with phase O (output projection) per 512-token half.
    if level < 5:
        _noop_out()
        return
    attnT = [p_at.tile([128, S], BF16, tag=f"at{h}", name=f"at{h}")
             for h in range(HPG)]

    def _qr(h):
        return qrP[h // 2][(h % 2) * 64:(h % 2) * 64 + 64, :]

    with tc.tile_pool(name="ap", bufs=2, space="PSUM") as ap_, \
         tc.tile_pool(name="sp", bufs=3, space="PSUM") as sp, \
         tc.tile_pool(name="dp", bufs=1, space="PSUM") as dp, \
         tc.tile_pool(name="as_", bufs=3) as as_, \
         tc.tile_pool(name="op", bufs=2, space="PSUM") as op_, \
         tc.tile_pool(name="os", bufs=3) as os_:
        for qt in range(NQA):
            q0 = qt * QTA
            fin_prev = None  # deferred normalization of the previous head

            def _finalize(acc, rd, h, q0=q0):
                rdp = sp.tile([128, QTA], F32, tag="s")
                nc.tensor.matmul(rdp[:], ones_r[:], rd[:],
                                 start=True, stop=True)
                rdb = as_.tile([128, QTA], F32, tag="rdb", bufs=2)
                nc.vector.tensor_copy(rdb[:], rdp[:])
                nc.vector.tensor_mul(attnT[h][:, q0:q0 + QTA],
                                     acc[:], rdb[:])

            for h in range(HPG):
                # apply the q-RMS scale for this q-tile, all heads
                nc.vector.tensor_mul(qnT[h][:, q0:q0 + QTA],
                                     qnT[h][:, q0:q0 + QTA],
                                     rq_bc[:, q0:q0 + QTA])
            for p in range(HPG // 2):
                nc.vector.tensor_mul(qrP[p][:, q0:q0 + QTA],
                                     qrP[p][:, q0:q0 + QTA],
                                     rq_bc[:, q0:q0 + QTA])
            for h in range(HPG):
                kbs = [kb for kb in range(NT) if block_cls[(kb, qt)] != SKIP]
                acc = ap_.tile([128, QTA], F32, tag="acc")
                den = dp.tile([1, QTA], F32, tag="den")
                esum = (as_.tile([128, QTA], BF16, tag="esum", bufs=2,
                                 name="esum")
                        if den_dve else None)
                nkb = len(kbs)
                pend = []  # software pipeline: delay av/den by two blocks

                def _flush(h=h, acc=acc, den=den, esum=esum, nkb=nkb):
                    pkb, pe, pi, plv, pw = pend.pop(0)
                    nc.tensor.matmul(acc[:, plv:plv + pw],
                                     v_sb[pkb][:, h * DV:(h + 1) * DV],
                                     pe[:, 0:pw], start=(pi == 0),
                                     stop=(pi == nkb - 1))
                    if den_dve:
                        if pi == 0:
                            nc.vector.tensor_copy(esum[:], pe[:])
                        else:
                            nc.vector.tensor_add(esum[:, plv:plv + pw],
                                                 esum[:, plv:plv + pw],
                                                 pe[:, 0:pw])
                        if pi == nkb - 1:
                            nc.tensor.matmul(den[:], ones[:, 0:1], esum[:],
                                             start=True, stop=True)
                    else:
                        nc.tensor.matmul(den[:, plv:plv + pw], ones[:, 0:1],
                                         pe[:, 0:pw],
                                         start=(pi == 0), stop=(pi == nkb - 1))

                for i, kb in enumerate(kbs):
                    # live q-suffix of this block: fully-masked leading
                    # columns are never computed (causal wedge).
                    ms = (mixed_slot.get((kb, qt))
                          if block_cls[(kb, qt)] == MIXED else None)
                    lv = ms[2] if (ms is not None and ms[0] == 'n'
                                   and i > 0) else 0
                    w = QTA - lv
                    ps = sp.tile([128, QTA], F32, tag="s")
                    nc.tensor.matmul(ps[:, 0:w],
                                     knT[h][:, kb * 128:(kb + 1) * 128],
                                     qnT[h][:, q0 + lv:q0 + QTA],
                                     start=True, stop=False)
                    b0 = (h % 2) * 64
                    nc.tensor.matmul(ps[:, 0:w],
                                     krT[b0:b0 + 64,
                                         kb * 128:(kb + 1) * 128],
                                     _qr(h)[:, q0 + lv:q0 + QTA],
                                     start=False, stop=True)
                    if i == min(4, nkb - 1) and fin_prev is not None:
                        _finalize(*fin_prev)
                        fin_prev = None
                    e = as_.tile([128, QTA], BF16, tag="e", bufs=5)
                    nc.scalar.activation(e[:, 0:w], ps[:, 0:w], AF.Exp,
                                         scale=SCALE)
                    if ms is not None:
                        if ms[0] == 'n':
                            _, sl, _, w0 = ms
                            m0 = nw * QTA + sl * 128
                            co = w0 - lv
                            nc.vector.tensor_mul(
                                e[:, co:co + 128], e[:, co:co + 128],
                                msk_sb[:, m0:m0 + 128])
                        else:
                            sl = ms[1]
                            nc.vector.tensor_mul(
                                e[:], e[:],
                                msk_sb[:, sl * QTA:(sl + 1) * QTA])
                    pend.append((kb, e, i, lv, w))
                    if len(pend) > 3:
                        _flush()
                while pend:
                    _flush()
                rd = as_.tile([1, QTA], F32R, tag="rd", bufs=2)
                with nc.allow_low_precision(reason="f32r broadcast operand"):
                    nc.vector.reciprocal(rd[:], den[:])
                fin_prev = (acc, rd, h)
            _finalize(*fin_prev)
            if level >= 6:
                # output projection for the finished 512-token half
                c0 = qt * QTA
                for mb in range(D // 128):
                    ps = op_.tile([128, 512], F32, tag="o")
                    for hc in range(HPG):
                        nc.tensor.matmul(
                            ps[:], wout_sb[hc][:, mb * 128:(mb + 1) * 128],
                            attnT[hc][:, c0:c0 + 512],
                            start=(hc == 0), stop=(hc == HPG - 1))
                    ot = os_.tile([128, 512], BF16, tag="ot")
                    nc.scalar.copy(ot[:], ps[:])
                    nc.scalar.dma_start(outT[mb * 128:(mb + 1) * 128,
                                             c0:c0 + 512], ot[:])
        if level < 6:
            _noop_out()


def _fix_multiwait(nc):
    """This container's walrus only supports ONE sem-wait per instruction.
    Hoist excess waits onto freshly inserted same-engine Drain instructions
    placed immediately before the owner (engine executes in order, so the
    AND-semantics of multiple waits is preserved)."""
    import bass_rust
    n = [0]
    for fn in nc.m.functions:
        for blk in fn.blocks:
            out, changed = [], False
            for inst in blk.instructions:
                si = inst.sync_info
                waits = list(si.on_wait) if (si is not None and si.on_wait) else []
                if len(waits) > 1:
                    changed = True
                    for w in waits[:-1]:
                        n[0] += 1
                        d = bass_rust.InstDrain(
                            name=f"MWFIX-{n[0]}", engine=inst.engine,
                            ins=[], outs=[])
                        d.sync_info = bass_rust.SyncInfo(on_wait=[w],
                                                         on_update=[])
                        out.append(d)
                    si.on_wait = [waits[-1]]
                    inst.sync_info = si
                out.append(inst)
            if changed:
                blk.instructions = out


# ======================= host-side preparation =======================

def _bf16(a):
    return np.asarray(a, np.float32).astype(ml_dtypes.bfloat16)


def rope_tables():
    inv_freq = 1.0 / THETA ** (np.arange(0, DR, 2, dtype=np.float32) / DR)
    pos = np.arange(S, dtype=np.float32)
    freqs = np.outer(pos, inv_freq)
    emb = np.concatenate([freqs, freqs], axis=-1)          # [S, 64]
    cos = np.cos(emb).astype(np.float32)
    sin = np.sin(emb).astype(np.float32)
    sin_s = sin.copy()
    sin_s[:, 0::2] *= -1.0
    return cos, sin_s


def perm_matrix():
    """perm[p, i] = 1 iff p == pairswap(i); symmetric. Block diag x2 for
    the packed 2-head qr tiles."""
    p64 = np.zeros((64, 64), np.float32)
    for i in range(0, 64, 2):
        p64[i + 1, i] = 1.0
        p64[i, i + 1] = 1.0
    out = np.zeros((128, 128), np.float32)
    out[:64, :64] = p64
    out[64:, 64:] = p64
    return out


def analyze_mask(mask):
    """mask: [1,1,S,S] additive. Returns block_cls + packed mask tiles.
    Mixed blocks store ('n', slot, live0, win0) when their fully-masked
    columns form a prefix [0, live0) and the partially-masked columns fit
    one 128-wide window at win0; others store ('w', slot) with the full
    [128, QTA] pattern. Identical patterns dedupe to one slot."""
    global _MASK_SLOTS, _MASK_NS, _MASK_NW
    m = np.asarray(mask, np.float32).reshape(S, S)          # [q, k]
    block_cls = {}
    slot_map = {}
    nar_of, nar = {}, []
    wid_of, wid = {}, []
    for qt in range(NQA):
        first_live = None
        for kb in range(NT):
            sub = m[qt * QTA:(qt + 1) * QTA, kb * 128:(kb + 1) * 128]  # [q,k]
            if np.all(sub <= -1e8):
                block_cls[(kb, qt)] = SKIP
                continue
            if first_live is None:
                first_live = kb
            if np.all(sub == 0.0):
                block_cls[(kb, qt)] = FREE
                continue
            block_cls[(kb, qt)] = MIXED
            t = (sub.T > -1e8).astype(np.float32)       # [k=128, q=QTA]
            anyok = np.any(t == 1.0, axis=0)
            allok = np.all(t == 1.0, axis=0)
            live0 = int(np.argmax(anyok))               # first col w/ any 1
            if kb == first_live:
                live0 = 0                               # start=True coverage
            prefix_dead = not np.any(anyok[:live0])
            bad = np.where(~allok)[0]
            bad = bad[bad >= live0]
            narrow = (prefix_dead and len(bad) > 0
                      and int(bad.max()) - int(bad.min()) < 128)
            if narrow:
                w0 = min(int(bad.min()), QTA - 128)
                pat = t[:, w0:w0 + 128]
                key = pat.tobytes()
                if key not in nar_of:
                    nar_of[key] = len(nar)
                    nar.append(pat)
                slot_map[(kb, qt)] = ('n', nar_of[key], live0, w0)
            else:
                key = t.tobytes()
                if key not in wid_of:
                    wid_of[key] = len(wid)
                    wid.append(t)
                slot_map[(kb, qt)] = ('w', wid_of[key])
    ns, nw = len(nar), len(wid)
    packed = np.zeros((128, max(nw * QTA + ns * 128, 128)), np.float32)
    for sl, t in enumerate(wid):
        packed[:, sl * QTA:(sl + 1) * QTA] = t
    for sl, pat in enumerate(nar):
        c = nw * QTA + sl * 128
        packed[:, c:c + 128] = pat
    _MASK_SLOTS, _MASK_NS, _MASK_NW = slot_map, ns, nw
    return block_cls, _bf16(packed)


def prep_core_inputs(inputs, wqa_cols=WQA_SL):
    """Returns (in_maps list of 8 dicts, block_cls)."""
    x = np.asarray(inputs["x"], np.float32)
    Wqa = np.asarray(inputs["Wqa"], np.float32)
    qw = np.asarray(inputs["q_a_norm_w"], np.float32)
    Wqb = np.asarray(inputs["Wqb"], np.float32)
    Wkva = np.asarray(inputs["Wkva"], np.float32)
    kvw = np.asarray(inputs["kv_a_norm_w"], np.float32)
    Wkvb = np.asarray(inputs["Wkvb"], np.float32)
    Wout = np.asarray(inputs["Wout"], np.float32)

    block_cls, packed = analyze_mask(inputs["attention_mask"])

    wq_eff = Wqa @ (qw[:, None] * Wqb)                      # [D, H*192]
    wq_eff = wq_eff.reshape(D, H, DN + DR)
    wkvb_w = kvw[:, None] * Wkvb                            # [RKV, H*256]
    wkvb_w = wkvb_w.reshape(RKV, H, DN + DV)
    wout_h = Wout.reshape(H, DV, D)

    cos, sin_s = rope_tables()
    cosq = _bf16(np.vstack([cos.T, cos.T]))                 # [128, S]
    sinq = _bf16(np.vstack([sin_s.T, sin_s.T]))
    permq = _bf16(perm_matrix())

    in_maps = []
    for c in range(NCORE):
        b, g = c // TP, c % TP
        hs = slice(g * HPG, (g + 1) * HPG)
        xt_b = _bf16(x[b].T.copy())
        t0 = g * (S // TP)
        m = {
            "xT": xt_b,
            "xkv": np.ascontiguousarray(xt_b[:, t0:t0 + S // TP]),
            "coskv": np.ascontiguousarray(cosq[:, t0:t0 + S // TP]),
            "sinkv": np.ascontiguousarray(sinq[:, t0:t0 + S // TP]),
            "wqa": _bf16(Wqa[:, g * wqa_cols:(g + 1) * wqa_cols]
                         if wqa_cols < RQ else Wqa),
            "wqn": _bf16(wq_eff[:, hs, :DN].reshape(D, HPG * DN)),
            "wqr": _bf16(wq_eff[:, hs, DN:].reshape(D, HPG * DR)),
            "wkva": _bf16(np.concatenate([Wkva, Wkva[:, RKV:]], axis=1)),
            "wkbk": _bf16(wkvb_w[:, hs, :DN].reshape(RKV, HPG * DN)),
            "wkbv": _bf16(wkvb_w[:, hs, DN:].reshape(RKV, HPG * DV)),
            "wout": _bf16(wout_h[hs].reshape(HPG * DV, D)),
            "cosq": cosq,
            "sinq": sinq,
            "perm": permq,
            "masks": packed,
        }
        in_maps.append(m)
    return in_maps, block_cls


def postprocess(results):
    """results: list of 8 dicts with 'outT' [D, S] bf16 partials."""
    out = np.empty((B, S, D), np.float32)
    for b in range(B):
        acc = results[b * TP]["outT"].astype(np.float32).copy()
        for g in range(1, TP):
            acc += results[b * TP + g]["outT"]
        out[b] = acc.T
    return out


# ======================= kernel entry point =======================

_program_cache = {}


def _mask_key(block_cls, packed):
    h = hashlib.sha256()
    h.update(repr(sorted(block_cls.items())).encode())
    h.update(repr(sorted(_MASK_SLOTS.items())).encode())
    h.update(np.ascontiguousarray(packed).tobytes())
    return h.hexdigest()


def kernel(**inputs):
    """Full-input MLA forward on 8 NeuronCores.

    Sharding: data-parallel over batch (2) x tensor-parallel over heads
    (4 groups of 4); the per-token q-RMS statistic is AllReduce'd inside
    each batch group. Host folds Wqa@Wqb, shards weights by head, casts to
    bf16 and transposes x; device returns per-core transposed partial
    outputs which the host sums per batch group.
    """
    from concourse.bass_utils import run_bass_kernel_spmd

    in_maps, block_cls = prep_core_inputs(inputs)
    n_mixed = sum(1 for v in block_cls.values() if v == MIXED)
    key = _mask_key(block_cls, in_maps[0]["masks"])
    nc = _program_cache.get(key)
    if nc is None:
        nc = build_program(block_cls, n_mixed, use_collective=True)
        _program_cache[key] = nc
    res = run_bass_kernel_spmd(nc, in_maps, core_ids=list(range(NCORE)))
    return postprocess(res.results)


# revision 26
# speedup vs baseline: 1.1678x; 1.1678x over previous
"""MLA (multi-head latent attention) Bass kernel for TRN2, 8-core SPMD.

Sharding: DP over batch (2) x TP over heads (4 groups of 4 heads).
core c: batch b = c // 4, head-group g = c % 4 (heads 4g..4g+3).

Math (per core), v2 (d-major everywhere, no PE transposes):
  kv_aT   = Wkva^T x^T  (d-major [576, S]); ssq_kv via ones@sq matmuls
  kvnT    = kv_aT[:512] * rsqrt(mean sq)    (broadcast via PE)
  krT     = rope(kv_aT[512:]) d-major       (pair-swap via perm matmul)
  ssq_q   = ones @ (Wqa_slice^T x^T)^2 ; AllReduce -> rq
  qnT     = Wqn^T x^T (d-major per head) ; qrT = rope(Wqr^T x^T) packed 2-head
  knT     = Wkbk^T kvnT ; v = kvnT^T Wkbv
  e[k,q]  = exp(SCALE * (qT . kT)) * tril-window  (128-wide diag mask only)
  attnT   = (v^T e) / (1^T e)                      per head
  outT    = Wout_g^T @ attnT                       partial over heads, host sums
"""

import copy
import functools
import hashlib
from contextlib import ExitStack
import numpy as np
import ml_dtypes

import concourse.bass as bass
import concourse.mybir as mybir
import concourse.tile as tile
from concourse.masks import make_identity

F32 = mybir.dt.float32
F32R = mybir.dt.float32r
BF16 = mybir.dt.bfloat16
AF = mybir.ActivationFunctionType

B, S, D = 2, 1024, 2048
H, DN, DR, DV = 16, 128, 64, 128
RQ, RKV = 1536, 512
THETA = 10000.0
EPS = 1e-6
SCALE = float((DN + DR) ** -0.5)

NCORE = 8
TP = 4                  # head groups
HPG = H // TP           # 4 heads per core
NT = S // 128           # 8 token blocks
NTH = 2                 # 512-token halves
QTA = 512               # attention q-tile width
NQA = S // QTA          # 2 attention q tiles
KC = D // 128           # 16 contraction chunks over D
RC = RKV // 128         # 4 contraction chunks over RKV
WQA_SL = RQ // TP       # 384 per-core Wqa column slice (for ssq)
QCC = WQA_SL // 128     # 3 ssq chunks

SKIP, FREE, MIXED = 0, 1, 2

# (kb, qt) -> ('n', slot, coff) | ('w', slot), set by analyze_mask;
# consumed by build_program in the same process.
_MASK_SLOTS = None
_MASK_NS = _MASK_NW = 0


def build_program(block_cls, n_mixed, use_collective=True, wqa_cols=WQA_SL,
                  trn_type="TRN2", fix_waits=True, reps=1, level=6,
                  use_kv_ag=True, den_dve=True):
    """block_cls: dict[(kb, qt)] -> SKIP/FREE/MIXED; mixed blocks get a
    binmask window from the packed `masks` input per _MASK_SLOTS."""
    nc = bass.Bass(trn_type, num_devices=NCORE if use_collective else 1)
    mixed_slot = dict(_MASK_SLOTS) if _MASK_SLOTS is not None else {}
    ns, nw = _MASK_NS, _MASK_NW

    xT = nc.dram_tensor("xT", [D, S], BF16, kind="ExternalInput")
    wqa = nc.dram_tensor("wqa", [D, wqa_cols], BF16, kind="ExternalInput")
    wqn = nc.dram_tensor("wqn", [D, HPG * DN], BF16, kind="ExternalInput")
    wqr = nc.dram_tensor("wqr", [D, HPG * DR], BF16, kind="ExternalInput")
    wkva = nc.dram_tensor("wkva", [D, RKV + 2 * DR], BF16,
                      kind="ExternalInput")
    wkbk = nc.dram_tensor("wkbk", [RKV, HPG * DN], BF16, kind="ExternalInput")
    wkbv = nc.dram_tensor("wkbv", [RKV, HPG * DV], BF16, kind="ExternalInput")
    wout = nc.dram_tensor("wout", [HPG * DV, D], BF16, kind="ExternalInput")
    xkv = nc.dram_tensor("xkv", [D, S // TP], BF16, kind="ExternalInput")
    coskv = nc.dram_tensor("coskv", [128, S // TP], BF16,
                           kind="ExternalInput")
    sinkv = nc.dram_tensor("sinkv", [128, S // TP], BF16,
                           kind="ExternalInput")
    cosq = nc.dram_tensor("cosq", [128, S], BF16, kind="ExternalInput")
    sinq = nc.dram_tensor("sinq", [128, S], BF16, kind="ExternalInput")
    perm = nc.dram_tensor("perm", [128, 128], BF16, kind="ExternalInput")
    masks = nc.dram_tensor("masks", [128, max(nw * QTA + ns * 128, 128)],
                           BF16, kind="ExternalInput")
    outT = nc.dram_tensor("outT", [D, S], BF16, kind="ExternalOutput")

    with tile.TileContext(nc) as tc:
        with tc.tile_pool(name="p_const", bufs=1) as p_const:
            ones = p_const.tile([128, 1], BF16, tag="ones")
            nc.gpsimd.memset(ones[:], 1.0)
            epsc = p_const.tile([128, 1], F32, tag="epsc")
            nc.gpsimd.memset(epsc[:], EPS)
            ones_f = p_const.tile([1, 128], F32, tag="ones_f")
            nc.gpsimd.memset(ones_f[:], 1.0)
            ones_r = p_const.tile([1, 128], F32R, tag="ones_r")
            with nc.allow_low_precision(reason="f32r broadcast operand"):
                nc.scalar.copy(ones_r[:], ones_f[:])
            consts = (ones, epsc, ones_f, ones_r)
            for _rep in range(reps):
                with ExitStack() as ctx:
                    _emit(ctx, nc, tc, locals(), use_collective, wqa_cols,
                          block_cls, mixed_slot, ns, nw, level=level,
                          consts=consts, use_kv_ag=use_kv_ag,
                          den_dve=den_dve)
    if fix_waits:
        _fix_multiwait(nc)
    return nc


def _emit(ctx, nc, tc, t, use_collective, wqa_cols, block_cls, mixed_slot,
          ns, nw, level=6, consts=None, use_kv_ag=True, den_dve=True):
    # level: timing-probe cutoff. 0=DMA only, 1=+KV, 2=+L, 3=+Q, 4=+KB/QS,
    # 5=+attention, 6=full (default). Levels <6 produce garbage outputs.
    xT, wqa, wqn, wqr, wkva, wkbk, wkbv, wout = (
        t["xT"], t["wqa"], t["wqn"], t["wqr"], t["wkva"], t["wkbk"],
        t["wkbv"], t["wout"])
    cosq, sinq, perm, masks, outT = (t["cosq"], t["sinq"], t["perm"],
                                     t["masks"], t["outT"])
    xkv, coskv, sinkv = t["xkv"], t["coskv"], t["sinkv"]
    ones, epsc, ones_f, ones_r = consts

    # ---------------- persistent pools ----------------
    p_x = ctx.enter_context(tc.tile_pool(name="p_x", bufs=1))
    p_tab = ctx.enter_context(tc.tile_pool(name="p_tab", bufs=1))
    p_qn = ctx.enter_context(tc.tile_pool(name="p_qn", bufs=1))
    p_qr = ctx.enter_context(tc.tile_pool(name="p_qr", bufs=1))
    p_kn = ctx.enter_context(tc.tile_pool(name="p_kn", bufs=1))
    p_kr = ctx.enter_context(tc.tile_pool(name="p_kr", bufs=1))
    p_v = ctx.enter_context(tc.tile_pool(name="p_v", bufs=1))
    p_kvn = ctx.enter_context(tc.tile_pool(name="p_kvn", bufs=1))
    p_at = ctx.enter_context(tc.tile_pool(name="p_at", bufs=1))
    p_rq = ctx.enter_context(tc.tile_pool(name="p_rq", bufs=1))
    p_msk = ctx.enter_context(tc.tile_pool(name="p_msk", bufs=1))
    p_dram = ctx.enter_context(tc.tile_pool(name="p_dram", bufs=1,
                                            space="DRAM"))
    # weight pools: opened for the whole body; loads are emitted in phase
    # order so the DMA engine streams them in priority order (prefetch).
    qw = ctx.enter_context(tc.tile_pool(name="qw", bufs=1))
    bw = ctx.enter_context(tc.tile_pool(name="bw", bufs=1))
    ow = ctx.enter_context(tc.tile_pool(name="ow", bufs=1))
    if use_kv_ag:
        # keep weight pools resident all rep: enables cross-rep prefetch
        early = None
        kxw = ctx.enter_context(tc.tile_pool(name="kxw", bufs=1))
        lw = ctx.enter_context(tc.tile_pool(name="lw", bufs=1))
    else:
        early = ExitStack()  # kxw+lw close after phase L (frees SBUF)
        kxw = early.enter_context(tc.tile_pool(name="kxw", bufs=1))
        lw = early.enter_context(tc.tile_pool(name="lw", bufs=1))

    # --- input DMAs, all on the SP queue in consumption order so each
    # phase's weights land before the phase starts: x+wkva (KV), wqa (L),
    # tables+perm (KV rope), wqn/wqr (Q), wkbk/wkbv (KB), masks (A),
    # wout (O).
    TQ = S // TP
    xt_sb, wkva_sb, xkv_sb = [], [], []
    if use_kv_ag:
        xkv_all = kxw.tile([128, KC * TQ], BF16, tag="xkv")
        nc.sync.dma_start(
            xkv_all[:].rearrange("p (c s) -> p c s", c=KC),
            xkv[:, :].rearrange("(c p) s -> p c s", p=128))
        xkv_sb = [xkv_all[:, kc * TQ:(kc + 1) * TQ] for kc in range(KC)]
        coskv_sb = p_tab.tile([128, TQ], BF16, tag="coskv")
        sinkv_sb = p_tab.tile([128, TQ], BF16, tag="sinkv")
        permkv_sb = p_tab.tile([128, 128], BF16, tag="permkv")
        nc.sync.dma_start(coskv_sb[:], coskv[:, :])
        nc.sync.dma_start(sinkv_sb[:], sinkv[:, :])
        nc.sync.dma_start(permkv_sb[:], perm[:, :])
    for i in range(4):
        wg = kxw.tile([128, 4 * (RKV + 2 * DR)], BF16, tag=f"wkva{i}")
        nc.sync.dma_start(
            wg[:].rearrange("p (c s) -> p c s", c=4),
            wkva[i * 512:(i + 1) * 512, :]
            .rearrange("(c p) s -> p c s", p=128))
        wkva_sb += [wg[:, j * (RKV + 2 * DR):(j + 1) * (RKV + 2 * DR)]
                    for j in range(4)]
    for i in range(4):
        xg = p_x.tile([128, 4 * S], BF16, tag=f"x{i}")
        nc.sync.dma_start(
            xg[:].rearrange("p (c s) -> p c s", c=4),
            xT[i * 512:(i + 1) * 512, :]
            .rearrange("(c p) s -> p c s", p=128))
        xt_sb += [xg[:, j * S:(j + 1) * S] for j in range(4)]

    cos_sb = p_tab.tile([128, S], BF16, tag="cos")
    sin_sb = p_tab.tile([128, S], BF16, tag="sin")
    perm_sb = p_tab.tile([128, 128], BF16, tag="perm")
    nc.sync.dma_start(cos_sb[:], cosq[:, :])
    nc.sync.dma_start(sin_sb[:], sinq[:, :])
    nc.sync.dma_start(perm_sb[:], perm[:, :])

    wqa_all = lw.tile([128, KC * wqa_cols], BF16, tag="wqa")
    wqa_sb = [wqa_all[:, kc * wqa_cols:(kc + 1) * wqa_cols]
              for kc in range(KC)]
    for i in range(4):
        nc.sync.dma_start(
            wqa_all[:, i * 4 * wqa_cols:(i + 1) * 4 * wqa_cols]
            .rearrange("p (c s) -> p c s", c=4),
            wqa[i * 512:(i + 1) * 512, :].rearrange("(c p) s -> p c s",
                                                    p=128))
    wqn_all = qw.tile([128, KC * HPG * DN], BF16, tag="wqn")
    wqr_all = qw.tile([128, KC * HPG * DR], BF16, tag="wqr")
    wqn_sb = [wqn_all[:, kc * HPG * DN:(kc + 1) * HPG * DN]
              for kc in range(KC)]
    wqr_sb = [wqr_all[:, kc * HPG * DR:(kc + 1) * HPG * DR]
              for kc in range(KC)]
    for i in range(4):
        nc.sync.dma_start(
            wqn_all[:, i * 4 * HPG * DN:(i + 1) * 4 * HPG * DN]
            .rearrange("p (c s) -> p c s", c=4),
            wqn[i * 512:(i + 1) * 512, :].rearrange("(c p) s -> p c s",
                                                    p=128))
        nc.sync.dma_start(
            wqr_all[:, i * 4 * HPG * DR:(i + 1) * 4 * HPG * DR]
            .rearrange("p (c s) -> p c s", c=4),
            wqr[i * 512:(i + 1) * 512, :].rearrange("(c p) s -> p c s",
                                                    p=128))
    wkbk_sb, wkbv_sb = [], []
    for rc in range(RC):
        wc = bw.tile([128, HPG * DN], BF16, tag=f"wkbk{rc}")
        nc.sync.dma_start(wc[:], wkbk[rc * 128:(rc + 1) * 128, :])
        wkbk_sb.append(wc)
        wc2 = bw.tile([128, HPG * DV], BF16, tag=f"wkbv{rc}")
        nc.sync.dma_start(wc2[:], wkbv[rc * 128:(rc + 1) * 128, :])
        wkbv_sb.append(wc2)
    mw = max(nw * QTA + ns * 128, 128)
    msk_sb = p_msk.tile([128, mw], BF16, tag="msk")
    nc.sync.dma_start(msk_sb[:], masks[:, :mw])
    wout_sb = []
    for hc in range(HPG):
        wc = ow.tile([128, D], BF16, tag=f"wo{hc}")
        nc.sync.dma_start(wc[:], wout[hc * 128:(hc + 1) * 128, :])
        wout_sb.append(wc)

    def _noop_out():
        # timing probes: still write outT so the program has its output
        for mb in range(D // 128):
            nc.sync.dma_start(outT[mb * 128:(mb + 1) * 128, 0:512],
                              xt_sb[0][:, 0:512])
        return

    # ------- phase KV: kv_aT d-major + rms + rope (no transposes) -------
    if level < 1:
        _noop_out()
        if early is not None:
            early.close()
        return
    kvnT = [p_kvn.tile([128, S], BF16, tag=f"kvn{rc}", name=f"kvn{rc}")
            for rc in range(RC)]
    krT = p_kr.tile([128, S], BF16, tag="krT")
    if use_kv_ag:
        # each core: its S/TP-token slice, fully normalized + roped, then
        # one AllGather of the packed [128, (RC+1)*TQ] result.
        kvag_in = p_dram.tile([128 * (RC + 1) * TQ], BF16, name="kvag_in")
        kvag_out = p_dram.tile([TP * 128 * (RC + 1) * TQ], BF16,
                               name="kvag_out")
        with tc.tile_pool(name="kp", bufs=3, space="PSUM") as kp, \
             tc.tile_pool(name="ksp", bufs=1, space="PSUM") as ksp, \
             tc.tile_pool(name="ks", bufs=2) as ks:
            kq = ks.tile([128, (RC + 1) * TQ], BF16, tag="kq", bufs=1)
            sqs = []
            for rc in range(RC):
                ps = kp.tile([128, TQ], F32, tag="kva")
                for kc in range(KC):
                    nc.tensor.matmul(ps[:],
                                     wkva_sb[kc][:, rc * 128:(rc + 1) * 128],
                                     xkv_sb[kc][:],
                                     start=(kc == 0), stop=(kc == KC - 1))
                nc.scalar.copy(kq[:, rc * TQ:(rc + 1) * TQ], ps[:])
                sq = ks.tile([128, TQ], BF16, tag=f"sq{rc}", bufs=1)
                nc.scalar.activation(sq[:], ps[:], AF.Square)
                sqs.append(sq)
            pr = kp.tile([128, TQ], F32, tag="kva")
            for kc in range(KC):
                nc.tensor.matmul(pr[:], wkva_sb[kc][:, RKV:], xkv_sb[kc][:],
                                 start=(kc == 0), stop=(kc == KC - 1))
            krq = ks.tile([128, TQ], BF16, tag="krq", bufs=1)
            nc.scalar.copy(krq[:], pr[:])
            sp_ = ksp.tile([1, TQ], F32, tag="ssq")
            for rc in range(RC):
                nc.tensor.matmul(sp_[:], ones[:, 0:1], sqs[rc][:],
                                 start=(rc == 0), stop=(rc == RC - 1))
            ssq_sb = ks.tile([1, TQ], F32, tag="ssq_sb", bufs=1)
            nc.vector.tensor_copy(ssq_sb[:], sp_[:])
            rk = ks.tile([1, TQ], F32, tag="rk", bufs=1)
            nc.scalar.activation(rk[:], ssq_sb[:], AF.Sqrt, scale=1.0 / RKV,
                                 bias=epsc[0:1, 0:1])
            sk = ks.tile([1, TQ], F32R, tag="sk", bufs=1)
            with nc.allow_low_precision(reason="f32r broadcast operand"):
                nc.vector.reciprocal(sk[:], rk[:])
            pb = kp.tile([128, TQ], F32, tag="kva")
            nc.tensor.matmul(pb[:], ones_r[:], sk[:], start=True, stop=True)
            skq = ks.tile([128, TQ], F32, tag="skq", bufs=1)
            nc.vector.tensor_copy(skq[:], pb[:])
            for rc in range(RC):
                nc.vector.tensor_mul(kq[:, rc * TQ:(rc + 1) * TQ],
                                     kq[:, rc * TQ:(rc + 1) * TQ], skq[:])
            # rope on the quarter
            m1 = ks.tile([128, TQ], BF16, tag="m1", bufs=1)
            nc.vector.tensor_mul(m1[:], krq[:], coskv_sb[:])
            prs = kp.tile([128, TQ], F32, tag="kva")
            nc.tensor.matmul(prs[:], permkv_sb[:], krq[:],
                             start=True, stop=True)
            t2 = ks.tile([128, TQ], F32, tag="t2", bufs=1)
            nc.vector.tensor_mul(t2[:], prs[:], sinkv_sb[:])
            nc.vector.tensor_add(kq[:, RC * TQ:(RC + 1) * TQ], m1[:], t2[:])
            nc.scalar.dma_start(
                kvag_in[:].rearrange("(p f) -> p f", p=128), kq[:])
        if use_collective:
            nc.gpsimd.collective_compute(
                "AllGather", mybir.AluOpType.bypass,
                replica_groups=[[0, 1, 2, 3], [4, 5, 6, 7]],
                ins=[kvag_in.opt()], outs=[kvag_out.opt()])

        def _kv_readback():
            # emitted after phase Q: the AllGather is long done by then, so
            # these triggers never stall the ACT stream.
            if not use_collective:
                for g in range(TP):
                    nc.scalar.dma_start(
                        kvag_out[g * 128 * (RC + 1) * TQ:
                                 (g + 1) * 128 * (RC + 1) * TQ],
                        kvag_in[:])
            for g in range(TP):
                view = kvag_out[g * 128 * (RC + 1) * TQ:
                                (g + 1) * 128 * (RC + 1) * TQ] \
                    .rearrange("(p f) -> p f", p=128)
                for rc in range(RC):
                    nc.scalar.dma_start(kvnT[rc][:, g * TQ:(g + 1) * TQ],
                                        view[:, rc * TQ:(rc + 1) * TQ])
                nc.scalar.dma_start(krT[:, g * TQ:(g + 1) * TQ],
                                    view[:, RC * TQ:(RC + 1) * TQ])
    else:
        _kv_readback = None
        krraw = p_kr.tile([128, S], BF16, tag="krraw")
        sk_bc = p_kr.tile([128, S], F32, tag="skbc")
        with tc.tile_pool(name="kp", bufs=2, space="PSUM") as kp, \
             tc.tile_pool(name="kp2", bufs=2, space="PSUM") as kp2, \
             tc.tile_pool(name="ksp", bufs=2, space="PSUM") as ksp, \
             tc.tile_pool(name="ks", bufs=2) as ks:
            ssq_ps = []
            for th in range(NTH):
                sqs = []
                for rc in range(RC):
                    ps = kp.tile([128, 512], F32, tag="kva")
                    for kc in range(KC):
                        nc.tensor.matmul(ps[:],
                                         wkva_sb[kc][:, rc * 128:(rc + 1) * 128],
                                         xt_sb[kc][:, th * 512:(th + 1) * 512],
                                         start=(kc == 0), stop=(kc == KC - 1))
                    nc.scalar.copy(kvnT[rc][:, th * 512:(th + 1) * 512], ps[:])
                    sq = ks.tile([128, 512], BF16, tag=f"sq{rc}", bufs=2)
                    nc.scalar.activation(sq[:], ps[:], AF.Square)
                    sqs.append(sq)
                pr = kp2.tile([128, 512], F32, tag="kvr")
                for kc in range(KC):
                    nc.tensor.matmul(pr[:], wkva_sb[kc][:, RKV:],
                                     xt_sb[kc][:, th * 512:(th + 1) * 512],
                                     start=(kc == 0), stop=(kc == KC - 1))
                nc.scalar.copy(krraw[:, th * 512:(th + 1) * 512], pr[:])
                sp_ = ksp.tile([1, 512], F32, tag="ssq")
                for rc in range(RC):
                    nc.tensor.matmul(sp_[:], ones[:, 0:1], sqs[rc][:],
                                     start=(rc == 0), stop=(rc == RC - 1))
                ssq_ps.append(sp_)
            # rsqrt chain on [1, S]
            ssq_sb = ks.tile([1, S], F32, tag="ssq_sb", bufs=1)
            for th in range(NTH):
                nc.vector.tensor_copy(ssq_sb[:, th * 512:(th + 1) * 512],
                                      ssq_ps[th][:])
            rk = ks.tile([1, S], F32, tag="rk", bufs=1)
            nc.scalar.activation(rk[:], ssq_sb[:], AF.Sqrt, scale=1.0 / RKV,
                                 bias=epsc[0:1, 0:1])
            sk = ks.tile([1, S], F32R, tag="sk", bufs=1)
            with nc.allow_low_precision(reason="f32r broadcast operand"):
                nc.vector.reciprocal(sk[:], rk[:])
            # broadcast to [128, S] via PE, then scale kvnT in place
            for th in range(NTH):
                pb = kp.tile([128, 512], F32, tag="kva")
                nc.tensor.matmul(pb[:], ones_r[:],
                                 sk[:, th * 512:(th + 1) * 512],
                                 start=True, stop=True)
                nc.vector.tensor_copy(sk_bc[:, th * 512:(th + 1) * 512],
                                      pb[:])
            for rc in range(RC):
                nc.vector.tensor_mul(kvnT[rc][:], kvnT[rc][:], sk_bc[:])
            # rope on krraw (d-major): krT = krraw*cos + (perm@krraw)*sin'
            m1 = ks.tile([128, S], BF16, tag="m1", bufs=1)
            nc.vector.tensor_mul(m1[:], krraw[:], cos_sb[:])
            for th in range(NTH):
                prs = kp2.tile([128, 512], F32, tag="prs", bufs=1)
                nc.tensor.matmul(prs[:], perm_sb[:],
                                 krraw[:, th * 512:(th + 1) * 512],
                                 start=True, stop=True)
                t2 = ks.tile([128, 512], F32, tag="t2")
                nc.vector.tensor_mul(t2[:], prs[:],
                                     sin_sb[:, th * 512:(th + 1) * 512])
                nc.vector.tensor_add(krT[:, th * 512:(th + 1) * 512],
                                     m1[:, th * 512:(th + 1) * 512], t2[:])

    # ---------------- phase L: q ssq (partial) + AllReduce ----------------
    if level < 2:
        _noop_out()
        if early is not None:
            early.close()
        return
    ssq_in = p_dram.tile([S], F32)
    ssq_out = p_dram.tile([S], F32)
    with tc.tile_pool(name="lp", bufs=2, space="PSUM") as lp, \
         tc.tile_pool(name="lsp", bufs=1, space="PSUM") as lsp, \
         tc.tile_pool(name="ls", bufs=2) as ls:
        qssq_sb = ls.tile([1, S], F32, tag="qssq", bufs=1)
        for th in range(NTH):
            sqs = []
            for cc in range(QCC):
                ps = lp.tile([128, 512], F32, tag="qa")
                for kc in range(KC):
                    nc.tensor.matmul(ps[:],
                                     wqa_sb[kc][:, cc * 128:(cc + 1) * 128],
                                     xt_sb[kc][:, th * 512:(th + 1) * 512],
                                     start=(kc == 0), stop=(kc == KC - 1))
                sq = ls.tile([128, 512], BF16, tag=f"qsq{cc}", bufs=2)
                nc.scalar.activation(sq[:], ps[:], AF.Square)
                sqs.append(sq)
            sp_ = lsp.tile([1, 512], F32, tag="qssqp", bufs=2)
            for cc in range(QCC):
                nc.tensor.matmul(sp_[:], ones[:, 0:1], sqs[cc][:],
                                 start=(cc == 0), stop=(cc == QCC - 1))
            nc.vector.tensor_copy(qssq_sb[:, th * 512:(th + 1) * 512],
                                  sp_[:])
        nc.scalar.dma_start(ssq_in[:].rearrange("(one s) -> one s",
                                                one=1), qssq_sb[:])
    if early is not None:
        early.close()
    if use_collective:
        nc.gpsimd.collective_compute(
            "AllReduce", mybir.AluOpType.add,
            replica_groups=[[0, 1, 2, 3], [4, 5, 6, 7]],
            ins=[ssq_in.opt()], outs=[ssq_out.opt()])
    else:
        nc.sync.dma_start(ssq_out[:], ssq_in[:])

    # ------- phase Q: qr (packed 2-head, d-major rope) + q_nope -------
    if level < 3:
        _noop_out()
        return
    qnT = [p_qn.tile([128, S], BF16, tag=f"qn{h}", name=f"qn{h}")
           for h in range(HPG)]
    qrP = [p_qr.tile([128, S], BF16, tag=f"qr{p}", name=f"qr{p}")
           for p in range(HPG // 2)]
    with tc.tile_pool(name="qp", bufs=2, space="PSUM") as qp, \
         tc.tile_pool(name="qpt", bufs=1, space="PSUM") as qpt, \
         tc.tile_pool(name="qs", bufs=2) as qs:
        qrraw = [qs.tile([128, S], BF16, tag=f"qrr{p}", bufs=1,
                         name=f"qrr{p}")
                 for p in range(HPG // 2)]
        for p in range(HPG // 2):
            for th in range(NTH):
                ps = qp.tile([128, 512], F32, tag="q", bufs=4)
                for kc in range(KC):
                    nc.tensor.matmul(ps[:],
                                     wqr_sb[kc][:, p * 128:(p + 1) * 128],
                                     xt_sb[kc][:, th * 512:(th + 1) * 512],
                                     start=(kc == 0), stop=(kc == KC - 1))
                nc.scalar.copy(qrraw[p][:, th * 512:(th + 1) * 512], ps[:])
        for h in range(HPG):
            for th in range(NTH):
                ps = qp.tile([128, 512], F32, tag="q", bufs=4)
                for kc in range(KC):
                    nc.tensor.matmul(
                        ps[:], wqn_sb[kc][:, h * DN:(h + 1) * DN],
                        xt_sb[kc][:, th * 512:(th + 1) * 512],
                        start=(kc == 0), stop=(kc == KC - 1))
                nc.scalar.copy(qnT[h][:, th * 512:(th + 1) * 512], ps[:])
        # d-major rope on packed qr pairs (DVE + perm matmuls)
        for p in range(HPG // 2):
            m1 = qs.tile([128, S], BF16, tag="qm1", bufs=2)
            nc.vector.tensor_mul(m1[:], qrraw[p][:], cos_sb[:])
            for th in range(NTH):
                prs = qpt.tile([128, 512], F32, tag="qprs", bufs=2)
                nc.tensor.matmul(prs[:], perm_sb[:],
                                 qrraw[p][:, th * 512:(th + 1) * 512],
                                 start=True, stop=True)
                t2 = qs.tile([128, 512], F32, tag="qt2", bufs=4)
                nc.vector.tensor_mul(t2[:], prs[:],
                                     sin_sb[:, th * 512:(th + 1) * 512])
                nc.vector.tensor_add(qrP[p][:, th * 512:(th + 1) * 512],
                                     m1[:, th * 512:(th + 1) * 512], t2[:])

    # rq recip chain + kv readback, emitted after phase Q so the r1 /
    # readback DMA triggers never stall the ACT stream on the collectives.
    rqs = ctx.enter_context(tc.tile_pool(name="rqs", bufs=1))
    r1 = rqs.tile([1, S], F32, tag="r1")
    nc.scalar.dma_start(r1[:],
                        ssq_out[:].rearrange("(one s) -> one s", one=1))
    r2 = rqs.tile([1, S], F32, tag="r2")
    nc.scalar.activation(r2[:], r1[:], AF.Sqrt, scale=1.0 / RQ,
                         bias=epsc[0:1, 0:1])
    r3 = rqs.tile([1, S], F32R, tag="r3")
    with nc.allow_low_precision(reason="f32r broadcast operand"):
        nc.vector.reciprocal(r3[:], r2[:])
    if _kv_readback is not None:
        _kv_readback()

    # ---------------- phase KB: k_nope (d-major) + v (token-major) ----------
    if level < 4:
        _noop_out()
        return
    knT = [p_kn.tile([128, S], BF16, tag=f"kn{h}", name=f"kn{h}")
           for h in range(HPG)]
    v_sb = [p_v.tile([128, HPG * DV], BF16, tag=f"v{tb}", name=f"v{tb}")
            for tb in range(NT)]
    with tc.tile_pool(name="bp", bufs=2, space="PSUM") as bp:
        for h in range(HPG):
            for th in range(NTH):
                ps = bp.tile([128, 512], F32, tag="kn")
                for rc in range(RC):
                    nc.tensor.matmul(
                        ps[:], wkbk_sb[rc][:, h * DN:(h + 1) * DN],
                        kvnT[rc][:, th * 512:(th + 1) * 512],
                        start=(rc == 0), stop=(rc == RC - 1))
                nc.scalar.copy(knT[h][:, th * 512:(th + 1) * 512], ps[:])
        for tb in range(NT):
            ps = bp.tile([128, HPG * DV], F32, tag="v")
            for rc in range(RC):
                nc.tensor.matmul(ps[:], kvnT[rc][:, tb * 128:(tb + 1) * 128],
                                 wkbv_sb[rc][:], start=(rc == 0),
                                 stop=(rc == RC - 1))
            nc.scalar.copy(v_sb[tb][:], ps[:])

    # ------- phase QS: broadcast 1/rms_q to [128, S] via PE -------
    rq_bc = p_rq.tile([128, S], F32, tag="rq")
    with tc.tile_pool(name="rqp", bufs=2, space="PSUM") as rqp:
        for th in range(NTH):
            pb = rqp.tile([128, 512], F32, tag="pb")
            nc.tensor.matmul(pb[:], ones_r[:],
                             r3[:, th * 512:(th + 1) * 512],
                             start=True, stop=True)
            nc.scalar.copy(rq_bc[:, th * 512:(th + 1) * 512], pb[:])

    # --- phase A: attention (transposed flash), software-pipelined, qt-major
    # interleaved with phase O (output projection) per 512-token half.
    if level < 5:
        _noop_out()
        return
    attnT = [p_at.tile([128, S], BF16, tag=f"at{h}", name=f"at{h}")
             for h in range(HPG)]

    def _qr(h):
        return qrP[h // 2][(h % 2) * 64:(h % 2) * 64 + 64, :]

    def _outproj(op_, os_, c0, on_dve=False):
        for mb in range(D // 128):
            ps = op_.tile([128, 512], F32, tag="o", name="o")
            for hc in range(HPG):
                nc.tensor.matmul(
                    ps[:], wout_sb[hc][:, mb * 128:(mb + 1) * 128],
                    attnT[hc][:, c0:c0 + 512],
                    start=(hc == 0), stop=(hc == HPG - 1))
            ot = os_.tile([128, 512], BF16, tag="ot", name="ot")
            if on_dve and mb % 2:
                nc.vector.tensor_copy(ot[:], ps[:])
            else:
                nc.scalar.copy(ot[:], ps[:])
            nc.scalar.dma_start(outT[mb * 128:(mb + 1) * 128,
                                     c0:c0 + 512], ot[:])

    att_pools = ExitStack()
    op_ = ctx.enter_context(tc.tile_pool(name="op", bufs=2, space="PSUM"))
    os_ = ctx.enter_context(tc.tile_pool(name="os", bufs=3))
    ap_ = att_pools.enter_context(tc.tile_pool(name="ap", bufs=2,
                                               space="PSUM"))
    sp = att_pools.enter_context(tc.tile_pool(name="sp", bufs=3,
                                              space="PSUM"))
    dp = att_pools.enter_context(tc.tile_pool(name="dp", bufs=1,
                                              space="PSUM"))
    as_ = att_pools.enter_context(tc.tile_pool(name="as_", bufs=3))
    if True:
        for qt in range(NQA):
            q0 = qt * QTA
            fin_prev = None  # deferred normalization of the previous head

            def _finalize(acc, rd, h, q0=q0):
                rdp = sp.tile([128, QTA], F32, tag="s")
                nc.tensor.matmul(rdp[:], ones_r[:], rd[:],
                                 start=True, stop=True)
                rdb = as_.tile([128, QTA], F32, tag="rdb", bufs=2)
                nc.vector.tensor_copy(rdb[:], rdp[:])
                nc.vector.tensor_mul(attnT[h][:, q0:q0 + QTA],
                                     acc[:], rdb[:])

            for h in range(HPG):
                # apply the q-RMS scale for this q-tile, all heads
                nc.vector.tensor_mul(qnT[h][:, q0:q0 + QTA],
                                     qnT[h][:, q0:q0 + QTA],
                                     rq_bc[:, q0:q0 + QTA])
            for p in range(HPG // 2):
                nc.vector.tensor_mul(qrP[p][:, q0:q0 + QTA],
                                     qrP[p][:, q0:q0 + QTA],
                                     rq_bc[:, q0:q0 + QTA])
            for h in range(HPG):
                kbs = [kb for kb in range(NT) if block_cls[(kb, qt)] != SKIP]
                acc = ap_.tile([128, QTA], F32, tag="acc")
                den = dp.tile([1, QTA], F32, tag="den")
                esum = (as_.tile([128, QTA], BF16, tag="esum", bufs=2,
                                 name="esum")
                        if den_dve else None)
                nkb = len(kbs)
                pend = []  # software pipeline: delay av/den by two blocks

                def _flush(h=h, acc=acc, den=den, esum=esum, nkb=nkb):
                    pkb, pe, pi, plv, pw = pend.pop(0)
                    nc.tensor.matmul(acc[:, plv:plv + pw],
                                     v_sb[pkb][:, h * DV:(h + 1) * DV],
                                     pe[:, 0:pw], start=(pi == 0),
                                     stop=(pi == nkb - 1))
                    if den_dve:
                        if pi == 0:
                            nc.vector.tensor_copy(esum[:], pe[:])
                        else:
                            nc.vector.tensor_add(esum[:, plv:plv + pw],
                                                 esum[:, plv:plv + pw],
                                                 pe[:, 0:pw])
                        if pi == nkb - 1:
                            nc.tensor.matmul(den[:], ones[:, 0:1], esum[:],
                                             start=True, stop=True)
                    else:
                        nc.tensor.matmul(den[:, plv:plv + pw], ones[:, 0:1],
                                         pe[:, 0:pw],
                                         start=(pi == 0), stop=(pi == nkb - 1))

                for i, kb in enumerate(kbs):
                    # live q-suffix of this block: fully-masked leading
                    # columns are never computed (causal wedge).
                    ms = (mixed_slot.get((kb, qt))
                          if block_cls[(kb, qt)] == MIXED else None)
                    lv = ms[2] if (ms is not None and ms[0] == 'n'
                                   and i > 0) else 0
                    w = QTA - lv
                    ps = sp.tile([128, QTA], F32, tag="s")
                    nc.tensor.matmul(ps[:, 0:w],
                                     knT[h][:, kb * 128:(kb + 1) * 128],
                                     qnT[h][:, q0 + lv:q0 + QTA],
                                     start=True, stop=False)
                    b0 = (h % 2) * 64
                    nc.tensor.matmul(ps[:, 0:w],
                                     krT[b0:b0 + 64,
                                         kb * 128:(kb + 1) * 128],
                                     _qr(h)[:, q0 + lv:q0 + QTA],
                                     start=False, stop=True)
                    if i == min(4, nkb - 1) and fin_prev is not None:
                        _finalize(*fin_prev)
                        fin_prev = None
                    e = as_.tile([128, QTA], BF16, tag="e", bufs=5)
                    nc.scalar.activation(e[:, 0:w], ps[:, 0:w], AF.Exp,
                                         scale=SCALE)
                    if ms is not None:
                        if ms[0] == 'n':
                            _, sl, _, w0 = ms
                            m0 = nw * QTA + sl * 128
                            co = w0 - lv
                            nc.vector.tensor_mul(
                                e[:, co:co + 128], e[:, co:co + 128],
                                msk_sb[:, m0:m0 + 128])
                        else:
                            sl = ms[1]
                            nc.vector.tensor_mul(
                                e[:], e[:],
                                msk_sb[:, sl * QTA:(sl + 1) * QTA])
                    pend.append((kb, e, i, lv, w))
                    if len(pend) > 3:
                        _flush()
                while pend:
                    _flush()
                rd = as_.tile([1, QTA], F32R, tag="rd", bufs=2)
                with nc.allow_low_precision(reason="f32r broadcast operand"):
                    nc.vector.reciprocal(rd[:], den[:])
                fin_prev = (acc, rd, h)
            _finalize(*fin_prev)
            if level >= 6 and qt < NQA - 1:
                _outproj(op_, os_, qt * QTA)
        if level < 6:
            _noop_out()
    att_pools.close()
    if level >= 6:
        # final half's projection runs with the attention pools freed so
        # the next rep's KV matmuls can claim PSUM banks immediately.
        _outproj(op_, os_, (NQA - 1) * QTA, on_dve=True)


def _fix_multiwait(nc):
    """This container's walrus only supports ONE sem-wait per instruction.
    Hoist excess waits onto freshly inserted same-engine Drain instructions
    placed immediately before the owner (engine executes in order, so the
    AND-semantics of multiple waits is preserved)."""
    import bass_rust
    n = [0]
    for fn in nc.m.functions:
        for blk in fn.blocks:
            out, changed = [], False
            for inst in blk.instructions:
                si = inst.sync_info
                waits = list(si.on_wait) if (si is not None and si.on_wait) else []
                if len(waits) > 1:
                    changed = True
                    for w in waits[:-1]:
                        n[0] += 1
                        d = bass_rust.InstDrain(
                            name=f"MWFIX-{n[0]}", engine=inst.engine,
                            ins=[], outs=[])
                        d.sync_info = bass_rust.SyncInfo(on_wait=[w],
                                                         on_update=[])
                        out.append(d)
                    si.on_wait = [waits[-1]]
                    inst.sync_info = si
                out.append(inst)
            if changed:
                blk.instructions = out


# ======================= host-side preparation =======================

def _bf16(a):
    return np.asarray(a, np.float32).astype(ml_dtypes.bfloat16)


def rope_tables():
    inv_freq = 1.0 / THETA ** (np.arange(0, DR, 2, dtype=np.float32) / DR)
    pos = np.arange(S, dtype=np.float32)
    freqs = np.outer(pos, inv_freq)
    emb = np.concatenate([freqs, freqs], axis=-1)          # [S, 64]
    cos = np.cos(emb).astype(np.float32)
    sin = np.sin(emb).astype(np.float32)
    sin_s = sin.copy()
    sin_s[:, 0::2] *= -1.0
    return cos, sin_s


def perm_matrix():
    """perm[p, i] = 1 iff p == pairswap(i); symmetric. Block diag x2 for
    the packed 2-head qr tiles."""
    p64 = np.zeros((64, 64), np.float32)
    for i in range(0, 64, 2):
        p64[i + 1, i] = 1.0
        p64[i, i + 1] = 1.0
    out = np.zeros((128, 128), np.float32)
    out[:64, :64] = p64
    out[64:, 64:] = p64
    return out


def analyze_mask(mask):
    """mask: [1,1,S,S] additive. Returns block_cls + packed mask tiles.
    Mixed blocks store ('n', slot, live0, win0) when their fully-masked
    columns form a prefix [0, live0) and the partially-masked columns fit
    one 128-wide window at win0; others store ('w', slot) with the full
    [128, QTA] pattern. Identical patterns dedupe to one slot."""
    global _MASK_SLOTS, _MASK_NS, _MASK_NW
    m = np.asarray(mask, np.float32).reshape(S, S)          # [q, k]
    block_cls = {}
    slot_map = {}
    nar_of, nar = {}, []
    wid_of, wid = {}, []
    for qt in range(NQA):
        first_live = None
        for kb in range(NT):
            sub = m[qt * QTA:(qt + 1) * QTA, kb * 128:(kb + 1) * 128]  # [q,k]
            if np.all(sub <= -1e8):
                block_cls[(kb, qt)] = SKIP
                continue
            if first_live is None:
                first_live = kb
            if np.all(sub == 0.0):
                block_cls[(kb, qt)] = FREE
                continue
            block_cls[(kb, qt)] = MIXED
            t = (sub.T > -1e8).astype(np.float32)       # [k=128, q=QTA]
            anyok = np.any(t == 1.0, axis=0)
            allok = np.all(t == 1.0, axis=0)
            live0 = int(np.argmax(anyok))               # first col w/ any 1
            if kb == first_live:
                live0 = 0                               # start=True coverage
            prefix_dead = not np.any(anyok[:live0])
            bad = np.where(~allok)[0]
            bad = bad[bad >= live0]
            narrow = (prefix_dead and len(bad) > 0
                      and int(bad.max()) - int(bad.min()) < 128)
            if narrow:
                w0 = min(int(bad.min()), QTA - 128)
                pat = t[:, w0:w0 + 128]
                key = pat.tobytes()
                if key not in nar_of:
                    nar_of[key] = len(nar)
                    nar.append(pat)
                slot_map[(kb, qt)] = ('n', nar_of[key], live0, w0)
            else:
                key = t.tobytes()
                if key not in wid_of:
                    wid_of[key] = len(wid)
                    wid.append(t)
                slot_map[(kb, qt)] = ('w', wid_of[key])
    ns, nw = len(nar), len(wid)
    packed = np.zeros((128, max(nw * QTA + ns * 128, 128)), np.float32)
    for sl, t in enumerate(wid):
        packed[:, sl * QTA:(sl + 1) * QTA] = t
    for sl, pat in enumerate(nar):
        c = nw * QTA + sl * 128
        packed[:, c:c + 128] = pat
    _MASK_SLOTS, _MASK_NS, _MASK_NW = slot_map, ns, nw
    return block_cls, _bf16(packed)


def prep_core_inputs(inputs, wqa_cols=WQA_SL):
    """Returns (in_maps list of 8 dicts, block_cls)."""
    x = np.asarray(inputs["x"], np.float32)
    Wqa = np.asarray(inputs["Wqa"], np.float32)
    qw = np.asarray(inputs["q_a_norm_w"], np.float32)
    Wqb = np.asarray(inputs["Wqb"], np.float32)
    Wkva = np.asarray(inputs["Wkva"], np.float32)
    kvw = np.asarray(inputs["kv_a_norm_w"], np.float32)
    Wkvb = np.asarray(inputs["Wkvb"], np.float32)
    Wout = np.asarray(inputs["Wout"], np.float32)

    block_cls, packed = analyze_mask(inputs["attention_mask"])

    wq_eff = Wqa @ (qw[:, None] * Wqb)                      # [D, H*192]
    wq_eff = wq_eff.reshape(D, H, DN + DR)
    wkvb_w = kvw[:, None] * Wkvb                            # [RKV, H*256]
    wkvb_w = wkvb_w.reshape(RKV, H, DN + DV)
    wout_h = Wout.reshape(H, DV, D)

    cos, sin_s = rope_tables()
    cosq = _bf16(np.vstack([cos.T, cos.T]))                 # [128, S]
    sinq = _bf16(np.vstack([sin_s.T, sin_s.T]))
    permq = _bf16(perm_matrix())

    in_maps = []
    for c in range(NCORE):
        b, g = c // TP, c % TP
        hs = slice(g * HPG, (g + 1) * HPG)
        xt_b = _bf16(x[b].T.copy())
        t0 = g * (S // TP)
        m = {
            "xT": xt_b,
            "xkv": np.ascontiguousarray(xt_b[:, t0:t0 + S // TP]),
            "coskv": np.ascontiguousarray(cosq[:, t0:t0 + S // TP]),
            "sinkv": np.ascontiguousarray(sinq[:, t0:t0 + S // TP]),
            "wqa": _bf16(Wqa[:, g * wqa_cols:(g + 1) * wqa_cols]
                         if wqa_cols < RQ else Wqa),
            "wqn": _bf16(wq_eff[:, hs, :DN].reshape(D, HPG * DN)),
            "wqr": _bf16(wq_eff[:, hs, DN:].reshape(D, HPG * DR)),
            "wkva": _bf16(np.concatenate([Wkva, Wkva[:, RKV:]], axis=1)),
            "wkbk": _bf16(wkvb_w[:, hs, :DN].reshape(RKV, HPG * DN)),
            "wkbv": _bf16(wkvb_w[:, hs, DN:].reshape(RKV, HPG * DV)),
            "wout": _bf16(wout_h[hs].reshape(HPG * DV, D)),
            "cosq": cosq,
            "sinq": sinq,
            "perm": permq,
            "masks": packed,
        }
        in_maps.append(m)
    return in_maps, block_cls


def postprocess(results):
    """results: list of 8 dicts with 'outT' [D, S] bf16 partials."""
    out = np.empty((B, S, D), np.float32)
    for b in range(B):
        acc = results[b * TP]["outT"].astype(np.float32).copy()
        for g in range(1, TP):
            acc += results[b * TP + g]["outT"]
        out[b] = acc.T
    return out


# ======================= kernel entry point =======================

_program_cache = {}


def _mask_key(block_cls, packed):
    h = hashlib.sha256()
    h.update(repr(sorted(block_cls.items())).encode())
    h.update(repr(sorted(_MASK_SLOTS.items())).encode())
    h.update(np.ascontiguousarray(packed).tobytes())
    return h.hexdigest()


def kernel(**inputs):
    """Full-input MLA forward on 8 NeuronCores.

    Sharding: data-parallel over batch (2) x tensor-parallel over heads
    (4 groups of 4); the per-token q-RMS statistic is AllReduce'd inside
    each batch group. Host folds Wqa@Wqb, shards weights by head, casts to
    bf16 and transposes x; device returns per-core transposed partial
    outputs which the host sums per batch group.
    """
    from concourse.bass_utils import run_bass_kernel_spmd

    in_maps, block_cls = prep_core_inputs(inputs)
    n_mixed = sum(1 for v in block_cls.values() if v == MIXED)
    key = _mask_key(block_cls, in_maps[0]["masks"])
    nc = _program_cache.get(key)
    if nc is None:
        nc = build_program(block_cls, n_mixed, use_collective=True)
        _program_cache[key] = nc
    res = run_bass_kernel_spmd(nc, in_maps, core_ids=list(range(NCORE)))
    return postprocess(res.results)


# revision 36
# speedup vs baseline: 1.8057x; 1.5462x over previous
"""MLA (multi-head latent attention) Bass kernel for TRN2, 8-core SPMD.

Sharding: DP over batch (2) x TP over heads (4 groups of 4 heads).
core c: batch b = c // 4, head-group g = c % 4 (heads 4g..4g+3).

Math (per core), v2 (d-major everywhere, no PE transposes):
  kv_aT   = Wkva^T x^T  (d-major [576, S]); ssq_kv via ones@sq matmuls
  kvnT    = kv_aT[:512] * rsqrt(mean sq)    (broadcast via PE)
  krT     = rope(kv_aT[512:]) d-major       (pair-swap via perm matmul)
  ssq_q   = ones @ (Wqa_slice^T x^T)^2 ; AllReduce -> rq
  qnT     = Wqn^T x^T (d-major per head) ; qrT = rope(Wqr^T x^T) packed 2-head
  knT     = Wkbk^T kvnT ; v = kvnT^T Wkbv
  e[k,q]  = exp(SCALE * (qT . kT)) * tril-window  (128-wide diag mask only)
  attnT   = (v^T e) / (1^T e)                      per head
  outT    = Wout_g^T @ attnT                       partial over heads, host sums
"""

import copy
import functools
import hashlib
from contextlib import ExitStack
import numpy as np
import ml_dtypes

import concourse.bass as bass
import concourse.mybir as mybir
import concourse.tile as tile
from concourse.masks import make_identity

F32 = mybir.dt.float32
F32R = mybir.dt.float32r
BF16 = mybir.dt.bfloat16
AF = mybir.ActivationFunctionType

B, S, D = 2, 1024, 2048
H, DN, DR, DV = 16, 128, 64, 128
RQ, RKV = 1536, 512
THETA = 10000.0
EPS = 1e-6
SCALE = float((DN + DR) ** -0.5)

NCORE = 8
TP = 4                  # head groups
HPG = H // TP           # 4 heads per core
NT = S // 128           # 8 token blocks
NTH = 2                 # 512-token halves
QTA = 512               # attention q-tile width
NQA = S // QTA          # 2 attention q tiles
KC = D // 128           # 16 contraction chunks over D
RC = RKV // 128         # 4 contraction chunks over RKV
WQA_SL = RQ // TP       # 384 per-core Wqa column slice (for ssq)
QCC = WQA_SL // 128     # 3 ssq chunks

SKIP, FREE, MIXED = 0, 1, 2

# (kb, qt) -> ('n', slot, coff) | ('w', slot), set by analyze_mask;
# consumed by build_program in the same process.
_MASK_SLOTS = None
_MASK_NS = _MASK_NW = 0


def build_program(block_cls, n_mixed, use_collective=True, wqa_cols=WQA_SL,
                  trn_type="TRN2", fix_waits=True, reps=1, level=6,
                  use_kv_ag=True, den_dve=False):
    """block_cls: dict[(kb, qt)] -> SKIP/FREE/MIXED; mixed blocks get a
    binmask window from the packed `masks` input per _MASK_SLOTS."""
    nc = bass.Bass(trn_type, num_devices=NCORE if use_collective else 1)
    mixed_slot = dict(_MASK_SLOTS) if _MASK_SLOTS is not None else {}
    ns, nw = _MASK_NS, _MASK_NW

    xT = nc.dram_tensor("xT", [D, S], BF16, kind="ExternalInput")
    wqa = nc.dram_tensor("wqa", [D, wqa_cols], BF16, kind="ExternalInput")
    wqn = nc.dram_tensor("wqn", [D, HPG * DN], BF16, kind="ExternalInput")
    wqr = nc.dram_tensor("wqr", [D, HPG * DR], BF16, kind="ExternalInput")
    wkva = nc.dram_tensor("wkva", [D, RKV + 2 * DR], BF16,
                      kind="ExternalInput")
    wkbk = nc.dram_tensor("wkbk", [RKV, HPG * DN], BF16, kind="ExternalInput")
    wkbv = nc.dram_tensor("wkbv", [RKV, HPG * DV], BF16, kind="ExternalInput")
    wout = nc.dram_tensor("wout", [HPG * DV, D], BF16, kind="ExternalInput")
    xkv = nc.dram_tensor("xkv", [D, S // TP], BF16, kind="ExternalInput")
    coskv = nc.dram_tensor("coskv", [128, S // TP], BF16,
                           kind="ExternalInput")
    sinkv = nc.dram_tensor("sinkv", [128, S // TP], BF16,
                           kind="ExternalInput")
    cosq = nc.dram_tensor("cosq", [128, S], BF16, kind="ExternalInput")
    sinq = nc.dram_tensor("sinq", [128, S], BF16, kind="ExternalInput")
    perm = nc.dram_tensor("perm", [128, 128], BF16, kind="ExternalInput")
    masks = nc.dram_tensor("masks", [128, max(nw * QTA + ns * 128, 128)],
                           BF16, kind="ExternalInput")
    outT = nc.dram_tensor("outT", [D, S], BF16, kind="ExternalOutput")

    with tile.TileContext(nc) as tc:
        with tc.tile_pool(name="p_const", bufs=1) as p_const:
            ones = p_const.tile([128, 1], BF16, tag="ones")
            nc.gpsimd.memset(ones[:], 1.0)
            epsc = p_const.tile([128, 1], F32, tag="epsc")
            nc.gpsimd.memset(epsc[:], EPS)
            ones_f = p_const.tile([1, 128], F32, tag="ones_f")
            nc.gpsimd.memset(ones_f[:], 1.0)
            ones_r = p_const.tile([1, 128], F32R, tag="ones_r")
            with nc.allow_low_precision(reason="f32r broadcast operand"):
                nc.scalar.copy(ones_r[:], ones_f[:])
            consts = (ones, epsc, ones_f, ones_r)
            for _rep in range(reps):
                with ExitStack() as ctx:
                    _emit(ctx, nc, tc, locals(), use_collective, wqa_cols,
                          block_cls, mixed_slot, ns, nw, level=level,
                          consts=consts, use_kv_ag=use_kv_ag,
                          den_dve=den_dve)
    if fix_waits:
        _fix_multiwait(nc)
    return nc


def _emit(ctx, nc, tc, t, use_collective, wqa_cols, block_cls, mixed_slot,
          ns, nw, level=6, consts=None, use_kv_ag=True, den_dve=False):
    # level: timing-probe cutoff. 0=DMA only, 1=+KV, 2=+L, 3=+Q, 4=+KB/QS,
    # 5=+attention, 6=full (default). Levels <6 produce garbage outputs.
    xT, wqa, wqn, wqr, wkva, wkbk, wkbv, wout = (
        t["xT"], t["wqa"], t["wqn"], t["wqr"], t["wkva"], t["wkbk"],
        t["wkbv"], t["wout"])
    cosq, sinq, perm, masks, outT = (t["cosq"], t["sinq"], t["perm"],
                                     t["masks"], t["outT"])
    xkv, coskv, sinkv = t["xkv"], t["coskv"], t["sinkv"]
    ones, epsc, ones_f, ones_r = consts

    # ---------------- persistent pools ----------------
    p_x = ctx.enter_context(tc.tile_pool(name="p_x", bufs=1))
    p_tab = ctx.enter_context(tc.tile_pool(name="p_tab", bufs=1))
    p_qn = ctx.enter_context(tc.tile_pool(name="p_qn", bufs=1))
    p_qr = ctx.enter_context(tc.tile_pool(name="p_qr", bufs=1))
    p_kn = ctx.enter_context(tc.tile_pool(name="p_kn", bufs=1))
    p_kr = ctx.enter_context(tc.tile_pool(name="p_kr", bufs=1))
    p_v = ctx.enter_context(tc.tile_pool(name="p_v", bufs=1))
    p_kvn = ctx.enter_context(tc.tile_pool(name="p_kvn", bufs=1))
    p_at = ctx.enter_context(tc.tile_pool(name="p_at", bufs=1))
    p_rq = ctx.enter_context(tc.tile_pool(name="p_rq", bufs=1))
    p_msk = ctx.enter_context(tc.tile_pool(name="p_msk", bufs=1))
    p_dram = ctx.enter_context(tc.tile_pool(name="p_dram", bufs=1,
                                            space="DRAM"))
    # weight pools: opened for the whole body; loads are emitted in phase
    # order so the DMA engine streams them in priority order (prefetch).
    qw = ctx.enter_context(tc.tile_pool(name="qw", bufs=1))
    bw = ctx.enter_context(tc.tile_pool(name="bw", bufs=1))
    ow = ctx.enter_context(tc.tile_pool(name="ow", bufs=1))
    if use_kv_ag:
        # keep weight pools resident all rep: enables cross-rep prefetch
        early = None
        kxw = ctx.enter_context(tc.tile_pool(name="kxw", bufs=1))
        lw = ctx.enter_context(tc.tile_pool(name="lw", bufs=1))
    else:
        early = ExitStack()  # kxw+lw close after phase L (frees SBUF)
        kxw = early.enter_context(tc.tile_pool(name="kxw", bufs=1))
        lw = early.enter_context(tc.tile_pool(name="lw", bufs=1))

    # --- input DMAs, all on the SP queue in consumption order so each
    # phase's weights land before the phase starts: x+wkva (KV), wqa (L),
    # tables+perm (KV rope), wqn/wqr (Q), wkbk/wkbv (KB), masks (A),
    # wout (O).
    TQ = S // TP
    xt_sb, wkva_sb, xkv_sb = [], [], []
    if use_kv_ag:
        xkv_all = kxw.tile([128, KC * TQ], BF16, tag="xkv")
        nc.sync.dma_start(
            xkv_all[:].rearrange("p (c s) -> p c s", c=KC),
            xkv[:, :].rearrange("(c p) s -> p c s", p=128))
        xkv_sb = [xkv_all[:, kc * TQ:(kc + 1) * TQ] for kc in range(KC)]
        coskv_sb = p_tab.tile([128, TQ], BF16, tag="coskv")
        sinkv_sb = p_tab.tile([128, TQ], BF16, tag="sinkv")
        permkv_sb = p_tab.tile([128, 128], BF16, tag="permkv")
        nc.sync.dma_start(coskv_sb[:], coskv[:, :])
        nc.sync.dma_start(sinkv_sb[:], sinkv[:, :])
        nc.sync.dma_start(permkv_sb[:], perm[:, :])
    for i in range(4):
        wg = kxw.tile([128, 4 * (RKV + 2 * DR)], BF16, tag=f"wkva{i}")
        nc.sync.dma_start(
            wg[:].rearrange("p (c s) -> p c s", c=4),
            wkva[i * 512:(i + 1) * 512, :]
            .rearrange("(c p) s -> p c s", p=128))
        wkva_sb += [wg[:, j * (RKV + 2 * DR):(j + 1) * (RKV + 2 * DR)]
                    for j in range(4)]
    for i in range(4):
        xg = p_x.tile([128, 4 * S], BF16, tag=f"x{i}")
        nc.sync.dma_start(
            xg[:].rearrange("p (c s) -> p c s", c=4),
            xT[i * 512:(i + 1) * 512, :]
            .rearrange("(c p) s -> p c s", p=128))
        xt_sb += [xg[:, j * S:(j + 1) * S] for j in range(4)]

    cos_sb = p_tab.tile([128, S], BF16, tag="cos")
    sin_sb = p_tab.tile([128, S], BF16, tag="sin")
    perm_sb = p_tab.tile([128, 128], BF16, tag="perm")
    nc.sync.dma_start(cos_sb[:], cosq[:, :])
    nc.sync.dma_start(sin_sb[:], sinq[:, :])
    nc.sync.dma_start(perm_sb[:], perm[:, :])

    wqa_all = lw.tile([128, KC * wqa_cols], BF16, tag="wqa")
    wqa_sb = [wqa_all[:, kc * wqa_cols:(kc + 1) * wqa_cols]
              for kc in range(KC)]
    for i in range(4):
        nc.sync.dma_start(
            wqa_all[:, i * 4 * wqa_cols:(i + 1) * 4 * wqa_cols]
            .rearrange("p (c s) -> p c s", c=4),
            wqa[i * 512:(i + 1) * 512, :].rearrange("(c p) s -> p c s",
                                                    p=128))
    wqn_all = qw.tile([128, KC * HPG * DN], BF16, tag="wqn")
    wqr_all = qw.tile([128, KC * HPG * DR], BF16, tag="wqr")
    wqn_sb = [wqn_all[:, kc * HPG * DN:(kc + 1) * HPG * DN]
              for kc in range(KC)]
    wqr_sb = [wqr_all[:, kc * HPG * DR:(kc + 1) * HPG * DR]
              for kc in range(KC)]
    for i in range(4):
        nc.sync.dma_start(
            wqn_all[:, i * 4 * HPG * DN:(i + 1) * 4 * HPG * DN]
            .rearrange("p (c s) -> p c s", c=4),
            wqn[i * 512:(i + 1) * 512, :].rearrange("(c p) s -> p c s",
                                                    p=128))
        nc.sync.dma_start(
            wqr_all[:, i * 4 * HPG * DR:(i + 1) * 4 * HPG * DR]
            .rearrange("p (c s) -> p c s", c=4),
            wqr[i * 512:(i + 1) * 512, :].rearrange("(c p) s -> p c s",
                                                    p=128))
    wkbk_sb, wkbv_sb = [], []
    for rc in range(RC):
        wc = bw.tile([128, HPG * DN], BF16, tag=f"wkbk{rc}")
        nc.sync.dma_start(wc[:], wkbk[rc * 128:(rc + 1) * 128, :])
        wkbk_sb.append(wc)
        wc2 = bw.tile([128, HPG * DV], BF16, tag=f"wkbv{rc}")
        nc.sync.dma_start(wc2[:], wkbv[rc * 128:(rc + 1) * 128, :])
        wkbv_sb.append(wc2)
    mw = max(nw * QTA + ns * 128, 128)
    msk_sb = p_msk.tile([128, mw], BF16, tag="msk")
    nc.sync.dma_start(msk_sb[:], masks[:, :mw])
    wout_sb = []
    for hc in range(HPG):
        wc = ow.tile([128, D], BF16, tag=f"wo{hc}")
        nc.sync.dma_start(wc[:], wout[hc * 128:(hc + 1) * 128, :])
        wout_sb.append(wc)

    def _noop_out():
        # timing probes: still write outT so the program has its output
        for mb in range(D // 128):
            nc.sync.dma_start(outT[mb * 128:(mb + 1) * 128, 0:512],
                              xt_sb[0][:, 0:512])
        return

    # ------- phase KV: kv_aT d-major + rms + rope (no transposes) -------
    if level < 1:
        _noop_out()
        if early is not None:
            early.close()
        return
    kvnT_all = p_kvn.tile([128, RC * S], BF16, tag="kvn")
    kvnT = [kvnT_all[:, rc * S:(rc + 1) * S] for rc in range(RC)]
    krT = p_kr.tile([128, S], BF16, tag="krT")
    if use_kv_ag:
        # kq stays in a persistent pool: its pending kvag-write DMA must
        # not block later pools recycling the same SBUF (WAR hazard).
        kq = p_kr.tile([128, (RC + 1) * (S // TP)], BF16, tag="kq")
        # each core: its S/TP-token slice, fully normalized + roped, then
        # one AllGather of the packed [128, (RC+1)*TQ] result.
        kvag_in = p_dram.tile([128 * (RC + 1) * TQ], BF16, name="kvag_in")
        kvag_out = p_dram.tile([TP * 128 * (RC + 1) * TQ], BF16,
                               name="kvag_out")
        with tc.tile_pool(name="kp", bufs=3, space="PSUM") as kp, \
             tc.tile_pool(name="ksp", bufs=1, space="PSUM") as ksp, \
             tc.tile_pool(name="ks", bufs=2) as ks:
            sqs = []
            for rc in range(RC):
                ps = kp.tile([128, TQ], F32, tag="kva")
                for kc in range(KC):
                    nc.tensor.matmul(ps[:],
                                     wkva_sb[kc][:, rc * 128:(rc + 1) * 128],
                                     xkv_sb[kc][:],
                                     start=(kc == 0), stop=(kc == KC - 1))
                nc.scalar.copy(kq[:, rc * TQ:(rc + 1) * TQ], ps[:])
                sq = ks.tile([128, TQ], BF16, tag=f"sq{rc}", bufs=1)
                nc.scalar.activation(sq[:], ps[:], AF.Square)
                sqs.append(sq)
            pr = kp.tile([128, TQ], F32, tag="kva")
            for kc in range(KC):
                nc.tensor.matmul(pr[:], wkva_sb[kc][:, RKV:], xkv_sb[kc][:],
                                 start=(kc == 0), stop=(kc == KC - 1))
            krq = ks.tile([128, TQ], BF16, tag="krq", bufs=1)
            nc.scalar.copy(krq[:], pr[:])
            sp_ = ksp.tile([1, TQ], F32, tag="ssq")
            for rc in range(RC):
                nc.tensor.matmul(sp_[:], ones[:, 0:1], sqs[rc][:],
                                 start=(rc == 0), stop=(rc == RC - 1))
            ssq_sb = ks.tile([1, TQ], F32, tag="ssq_sb", bufs=1)
            nc.vector.tensor_copy(ssq_sb[:], sp_[:])
            rk = ks.tile([1, TQ], F32, tag="rk", bufs=1)
            nc.scalar.activation(rk[:], ssq_sb[:], AF.Sqrt, scale=1.0 / RKV,
                                 bias=epsc[0:1, 0:1])
            sk = ks.tile([1, TQ], F32R, tag="sk", bufs=1)
            with nc.allow_low_precision(reason="f32r broadcast operand"):
                nc.vector.reciprocal(sk[:], rk[:])
            pb = kp.tile([128, TQ], F32, tag="kva")
            nc.tensor.matmul(pb[:], ones_r[:], sk[:], start=True, stop=True)
            skq = ks.tile([128, TQ], F32, tag="skq", bufs=1)
            nc.vector.tensor_copy(skq[:], pb[:])
            for rc in range(RC):
                nc.vector.tensor_mul(kq[:, rc * TQ:(rc + 1) * TQ],
                                     kq[:, rc * TQ:(rc + 1) * TQ], skq[:])
            # rope on the quarter
            m1 = ks.tile([128, TQ], BF16, tag="m1", bufs=1)
            nc.vector.tensor_mul(m1[:], krq[:], coskv_sb[:])
            prs = kp.tile([128, TQ], F32, tag="kva")
            nc.tensor.matmul(prs[:], permkv_sb[:], krq[:],
                             start=True, stop=True)
            t2 = ks.tile([128, TQ], F32, tag="t2", bufs=1)
            nc.vector.tensor_mul(t2[:], prs[:], sinkv_sb[:])
            nc.vector.tensor_add(kq[:, RC * TQ:(RC + 1) * TQ], m1[:], t2[:])
            nc.gpsimd.dma_start(
                kvag_in[:].rearrange("(p f) -> p f", p=128), kq[:])
        if use_collective:
            nc.gpsimd.collective_compute(
                "AllGather", mybir.AluOpType.bypass,
                replica_groups=[[0, 1, 2, 3], [4, 5, 6, 7]],
                ins=[kvag_in.opt()], outs=[kvag_out.opt()])

        def _kv_readback():
            # on the gpsimd queue, emitted right after the AllGather: the
            # triggers wait only on the collective, never stalling ACT.
            if not use_collective:
                for g in range(TP):
                    nc.gpsimd.dma_start(
                        kvag_out[g * 128 * (RC + 1) * TQ:
                                 (g + 1) * 128 * (RC + 1) * TQ],
                        kvag_in[:])
            for g in range(TP):
                view = kvag_out[g * 128 * (RC + 1) * TQ:
                                (g + 1) * 128 * (RC + 1) * TQ] \
                    .rearrange("(p f) -> p f", p=128)
                nc.gpsimd.dma_start(
                    kvnT_all[:].rearrange("p (c s) -> p c s", c=RC)
                    [:, :, g * TQ:(g + 1) * TQ],
                    view[:, 0:RC * TQ].rearrange("p (c s) -> p c s", c=RC))
                nc.gpsimd.dma_start(krT[:, g * TQ:(g + 1) * TQ],
                                    view[:, RC * TQ:(RC + 1) * TQ])
        _kv_readback()
    else:
        pass
        krraw = p_kr.tile([128, S], BF16, tag="krraw")
        sk_bc = p_kr.tile([128, S], F32, tag="skbc")
        with tc.tile_pool(name="kp", bufs=2, space="PSUM") as kp, \
             tc.tile_pool(name="kp2", bufs=2, space="PSUM") as kp2, \
             tc.tile_pool(name="ksp", bufs=2, space="PSUM") as ksp, \
             tc.tile_pool(name="ks", bufs=2) as ks:
            ssq_ps = []
            for th in range(NTH):
                sqs = []
                for rc in range(RC):
                    ps = kp.tile([128, 512], F32, tag="kva")
                    for kc in range(KC):
                        nc.tensor.matmul(ps[:],
                                         wkva_sb[kc][:, rc * 128:(rc + 1) * 128],
                                         xt_sb[kc][:, th * 512:(th + 1) * 512],
                                         start=(kc == 0), stop=(kc == KC - 1))
                    nc.scalar.copy(kvnT[rc][:, th * 512:(th + 1) * 512], ps[:])
                    sq = ks.tile([128, 512], BF16, tag=f"sq{rc}", bufs=2)
                    nc.scalar.activation(sq[:], ps[:], AF.Square)
                    sqs.append(sq)
                pr = kp2.tile([128, 512], F32, tag="kvr")
                for kc in range(KC):
                    nc.tensor.matmul(pr[:], wkva_sb[kc][:, RKV:],
                                     xt_sb[kc][:, th * 512:(th + 1) * 512],
                                     start=(kc == 0), stop=(kc == KC - 1))
                nc.scalar.copy(krraw[:, th * 512:(th + 1) * 512], pr[:])
                sp_ = ksp.tile([1, 512], F32, tag="ssq")
                for rc in range(RC):
                    nc.tensor.matmul(sp_[:], ones[:, 0:1], sqs[rc][:],
                                     start=(rc == 0), stop=(rc == RC - 1))
                ssq_ps.append(sp_)
            # rsqrt chain on [1, S]
            ssq_sb = ks.tile([1, S], F32, tag="ssq_sb", bufs=1)
            for th in range(NTH):
                nc.vector.tensor_copy(ssq_sb[:, th * 512:(th + 1) * 512],
                                      ssq_ps[th][:])
            rk = ks.tile([1, S], F32, tag="rk", bufs=1)
            nc.scalar.activation(rk[:], ssq_sb[:], AF.Sqrt, scale=1.0 / RKV,
                                 bias=epsc[0:1, 0:1])
            sk = ks.tile([1, S], F32R, tag="sk", bufs=1)
            with nc.allow_low_precision(reason="f32r broadcast operand"):
                nc.vector.reciprocal(sk[:], rk[:])
            # broadcast to [128, S] via PE, then scale kvnT in place
            for th in range(NTH):
                pb = kp.tile([128, 512], F32, tag="kva")
                nc.tensor.matmul(pb[:], ones_r[:],
                                 sk[:, th * 512:(th + 1) * 512],
                                 start=True, stop=True)
                nc.vector.tensor_copy(sk_bc[:, th * 512:(th + 1) * 512],
                                      pb[:])
            for rc in range(RC):
                nc.vector.tensor_mul(kvnT[rc][:], kvnT[rc][:], sk_bc[:])
            # rope on krraw (d-major): krT = krraw*cos + (perm@krraw)*sin'
            m1 = ks.tile([128, S], BF16, tag="m1", bufs=1)
            nc.vector.tensor_mul(m1[:], krraw[:], cos_sb[:])
            for th in range(NTH):
                prs = kp2.tile([128, 512], F32, tag="prs", bufs=1)
                nc.tensor.matmul(prs[:], perm_sb[:],
                                 krraw[:, th * 512:(th + 1) * 512],
                                 start=True, stop=True)
                t2 = ks.tile([128, 512], F32, tag="t2")
                nc.vector.tensor_mul(t2[:], prs[:],
                                     sin_sb[:, th * 512:(th + 1) * 512])
                nc.vector.tensor_add(krT[:, th * 512:(th + 1) * 512],
                                     m1[:, th * 512:(th + 1) * 512], t2[:])

    # ---------------- phase L: q ssq (partial) + AllReduce ----------------
    if level < 2:
        _noop_out()
        if early is not None:
            early.close()
        return
    ssq_in = p_dram.tile([S], F32)
    ssq_out = p_dram.tile([S], F32)
    with tc.tile_pool(name="lp", bufs=2, space="PSUM") as lp, \
         tc.tile_pool(name="lsp", bufs=1, space="PSUM") as lsp, \
         tc.tile_pool(name="ls", bufs=2) as ls:
        qssq_sb = ls.tile([1, S], F32, tag="qssq", bufs=1)
        for th in range(NTH):
            sqs = []
            for cc in range(QCC):
                ps = lp.tile([128, 512], F32, tag="qa")
                for kc in range(KC):
                    nc.tensor.matmul(ps[:],
                                     wqa_sb[kc][:, cc * 128:(cc + 1) * 128],
                                     xt_sb[kc][:, th * 512:(th + 1) * 512],
                                     start=(kc == 0), stop=(kc == KC - 1))
                sq = ls.tile([128, 512], BF16, tag=f"qsq{cc}", bufs=2)
                nc.scalar.activation(sq[:], ps[:], AF.Square)
                sqs.append(sq)
            sp_ = lsp.tile([1, 512], F32, tag="qssqp", bufs=2)
            for cc in range(QCC):
                nc.tensor.matmul(sp_[:], ones[:, 0:1], sqs[cc][:],
                                 start=(cc == 0), stop=(cc == QCC - 1))
            nc.vector.tensor_copy(qssq_sb[:, th * 512:(th + 1) * 512],
                                  sp_[:])
        nc.gpsimd.dma_start(ssq_in[:].rearrange("(one s) -> one s",
                                                one=1), qssq_sb[:])
    if early is not None:
        early.close()
    if use_collective:
        nc.gpsimd.collective_compute(
            "AllReduce", mybir.AluOpType.add,
            replica_groups=[[0, 1, 2, 3], [4, 5, 6, 7]],
            ins=[ssq_in.opt()], outs=[ssq_out.opt()])
    else:
        nc.sync.dma_start(ssq_out[:], ssq_in[:])

    # ------- phase Q: qr (packed 2-head, d-major rope) + q_nope -------
    if level < 3:
        _noop_out()
        return
    qnT = [p_qn.tile([128, S], BF16, tag=f"qn{h}", name=f"qn{h}")
           for h in range(HPG)]
    qrP = [p_qr.tile([128, S], BF16, tag=f"qr{p}", name=f"qr{p}")
           for p in range(HPG // 2)]
    with tc.tile_pool(name="qp", bufs=2, space="PSUM") as qp, \
         tc.tile_pool(name="qpt", bufs=1, space="PSUM") as qpt, \
         tc.tile_pool(name="qs", bufs=2) as qs:
        qrraw = [qs.tile([128, S], BF16, tag=f"qrr{p}", bufs=1,
                         name=f"qrr{p}")
                 for p in range(HPG // 2)]
        for p in range(HPG // 2):
            for th in range(NTH):
                ps = qp.tile([128, 512], F32, tag="q", bufs=4)
                for kc in range(KC):
                    nc.tensor.matmul(ps[:],
                                     wqr_sb[kc][:, p * 128:(p + 1) * 128],
                                     xt_sb[kc][:, th * 512:(th + 1) * 512],
                                     start=(kc == 0), stop=(kc == KC - 1))
                nc.scalar.copy(qrraw[p][:, th * 512:(th + 1) * 512], ps[:])
        for h in range(HPG):
            for th in range(NTH):
                ps = qp.tile([128, 512], F32, tag="q", bufs=4)
                for kc in range(KC):
                    nc.tensor.matmul(
                        ps[:], wqn_sb[kc][:, h * DN:(h + 1) * DN],
                        xt_sb[kc][:, th * 512:(th + 1) * 512],
                        start=(kc == 0), stop=(kc == KC - 1))
                nc.scalar.copy(qnT[h][:, th * 512:(th + 1) * 512], ps[:])
        # d-major rope on packed qr pairs (DVE + perm matmuls)
        for p in range(HPG // 2):
            m1 = qs.tile([128, S], BF16, tag="qm1", bufs=2)
            nc.vector.tensor_mul(m1[:], qrraw[p][:], cos_sb[:])
            for th in range(NTH):
                prs = qpt.tile([128, 512], F32, tag="qprs", bufs=2)
                nc.tensor.matmul(prs[:], perm_sb[:],
                                 qrraw[p][:, th * 512:(th + 1) * 512],
                                 start=True, stop=True)
                t2 = qs.tile([128, 512], F32, tag="qt2", bufs=4)
                nc.vector.tensor_mul(t2[:], prs[:],
                                     sin_sb[:, th * 512:(th + 1) * 512])
                nc.vector.tensor_add(qrP[p][:, th * 512:(th + 1) * 512],
                                     m1[:, th * 512:(th + 1) * 512], t2[:])

    # rq recip chain + kv readback, emitted after phase Q so the r1 /
    # readback DMA triggers never stall the ACT stream on the collectives.
    rqs = ctx.enter_context(tc.tile_pool(name="rqs", bufs=1))
    r1 = rqs.tile([1, S], F32, tag="r1")
    nc.scalar.dma_start(r1[:],
                        ssq_out[:].rearrange("(one s) -> one s", one=1))
    r2 = rqs.tile([1, S], F32, tag="r2")
    nc.scalar.activation(r2[:], r1[:], AF.Sqrt, scale=1.0 / RQ,
                         bias=epsc[0:1, 0:1])
    r3 = rqs.tile([1, S], F32R, tag="r3")
    with nc.allow_low_precision(reason="f32r broadcast operand"):
        nc.vector.reciprocal(r3[:], r2[:])

    # ---------------- phase KB: k_nope (d-major) + v (token-major) ----------
    if level < 4:
        _noop_out()
        return
    knT = [p_kn.tile([128, S], BF16, tag=f"kn{h}", name=f"kn{h}")
           for h in range(HPG)]
    v_sb = [p_v.tile([128, HPG * DV], BF16, tag=f"v{tb}", name=f"v{tb}")
            for tb in range(NT)]
    with tc.tile_pool(name="bp", bufs=2, space="PSUM") as bp:
        for h in range(HPG):
            for th in range(NTH):
                ps = bp.tile([128, 512], F32, tag="kn")
                for rc in range(RC):
                    nc.tensor.matmul(
                        ps[:], wkbk_sb[rc][:, h * DN:(h + 1) * DN],
                        kvnT[rc][:, th * 512:(th + 1) * 512],
                        start=(rc == 0), stop=(rc == RC - 1))
                if th % 2:
                    nc.vector.tensor_copy(
                        knT[h][:, th * 512:(th + 1) * 512], ps[:])
                else:
                    nc.scalar.copy(knT[h][:, th * 512:(th + 1) * 512],
                                   ps[:])
        for tb in range(NT):
            ps = bp.tile([128, HPG * DV], F32, tag="v")
            for rc in range(RC):
                nc.tensor.matmul(ps[:], kvnT[rc][:, tb * 128:(tb + 1) * 128],
                                 wkbv_sb[rc][:], start=(rc == 0),
                                 stop=(rc == RC - 1))
            if tb % 2:
                nc.vector.tensor_copy(v_sb[tb][:], ps[:])
            else:
                nc.scalar.copy(v_sb[tb][:], ps[:])

    # ------- phase QS: broadcast 1/rms_q to [128, S] via PE -------
    rq_bc = p_rq.tile([128, S], F32, tag="rq")
    with tc.tile_pool(name="rqp", bufs=2, space="PSUM") as rqp:
        for th in range(NTH):
            pb = rqp.tile([128, 512], F32, tag="pb")
            nc.tensor.matmul(pb[:], ones_r[:],
                             r3[:, th * 512:(th + 1) * 512],
                             start=True, stop=True)
            nc.vector.tensor_copy(rq_bc[:, th * 512:(th + 1) * 512], pb[:])

    # --- phase A: attention (transposed flash), software-pipelined, qt-major
    # interleaved with phase O (output projection) per 512-token half.
    if level < 5:
        _noop_out()
        return
    attnT = [p_at.tile([128, S], BF16, tag=f"at{h}", name=f"at{h}")
             for h in range(HPG)]

    def _qr(h):
        return qrP[h // 2][(h % 2) * 64:(h % 2) * 64 + 64, :]

    def _outproj(op_, os_, c0, on_dve=False):
        for mb0 in range(0, D // 128, 2):
            ot = os_.tile([128, 2 * 512], BF16, tag="ot", name="ot",
                          bufs=2)
            for mi in range(2):
                mb = mb0 + mi
                ps = op_.tile([128, 512], F32, tag="o", name="o")
                for hc in range(HPG):
                    nc.tensor.matmul(
                        ps[:], wout_sb[hc][:, mb * 128:(mb + 1) * 128],
                        attnT[hc][:, c0:c0 + 512],
                        start=(hc == 0), stop=(hc == HPG - 1))
                if on_dve == 'all' or (on_dve and mi % 2):
                    nc.vector.tensor_copy(
                        ot[:, mi * 512:(mi + 1) * 512], ps[:])
                else:
                    nc.scalar.copy(ot[:, mi * 512:(mi + 1) * 512], ps[:])
            nc.scalar.dma_start(
                outT[mb0 * 128:(mb0 + 2) * 128, c0:c0 + 512]
                .rearrange("(c p) s -> p c s", p=128),
                ot[:].rearrange("p (c s) -> p c s", c=2))

    att_pools = ExitStack()
    op_ = ctx.enter_context(tc.tile_pool(name="op", bufs=2, space="PSUM"))
    os_ = ctx.enter_context(tc.tile_pool(name="os", bufs=3))
    ap_ = att_pools.enter_context(tc.tile_pool(name="ap", bufs=2,
                                               space="PSUM"))
    sp = att_pools.enter_context(tc.tile_pool(name="sp", bufs=3,
                                              space="PSUM"))
    dp = att_pools.enter_context(tc.tile_pool(name="dp", bufs=1,
                                              space="PSUM"))
    as_ = att_pools.enter_context(tc.tile_pool(name="as_", bufs=3))
    if True:
        for qt in range(NQA):
            q0 = qt * QTA
            fin_prev = None  # deferred normalization of the previous head

            def _finalize(acc, rd, h, q0=q0):
                rdp = sp.tile([128, QTA], F32, tag="s")
                nc.tensor.matmul(rdp[:], ones_r[:], rd[:],
                                 start=True, stop=True)
                rdb = as_.tile([128, QTA], F32, tag="rdb", bufs=2)
                nc.vector.tensor_copy(rdb[:], rdp[:])
                nc.vector.tensor_mul(attnT[h][:, q0:q0 + QTA],
                                     acc[:], rdb[:])

            for h in range(HPG):
                # apply the q-RMS scale for this q-tile, all heads
                nc.vector.tensor_mul(qnT[h][:, q0:q0 + QTA],
                                     qnT[h][:, q0:q0 + QTA],
                                     rq_bc[:, q0:q0 + QTA])
            for p in range(HPG // 2):
                nc.vector.tensor_mul(qrP[p][:, q0:q0 + QTA],
                                     qrP[p][:, q0:q0 + QTA],
                                     rq_bc[:, q0:q0 + QTA])
            for h in range(HPG):
                kbs = [kb for kb in range(NT) if block_cls[(kb, qt)] != SKIP]
                acc = ap_.tile([128, QTA], F32, tag="acc")
                den = dp.tile([1, QTA], F32, tag="den")
                esum = (as_.tile([128, QTA], BF16, tag="esum", bufs=2,
                                 name="esum")
                        if den_dve else None)
                nkb = len(kbs)
                pend = []  # software pipeline: delay av/den by two blocks

                def _flush(h=h, acc=acc, den=den, esum=esum, nkb=nkb):
                    pkb, pe, pi, plv, pw = pend.pop(0)
                    nc.tensor.matmul(acc[:, plv:plv + pw],
                                     v_sb[pkb][:, h * DV:(h + 1) * DV],
                                     pe[:, 0:pw], start=(pi == 0),
                                     stop=(pi == nkb - 1))
                    if den_dve:
                        if pi == 0:
                            nc.vector.tensor_copy(esum[:], pe[:])
                        else:
                            nc.vector.tensor_add(esum[:, plv:plv + pw],
                                                 esum[:, plv:plv + pw],
                                                 pe[:, 0:pw])
                        if pi == nkb - 1:
                            nc.tensor.matmul(den[:], ones[:, 0:1], esum[:],
                                             start=True, stop=True)
                    else:
                        nc.tensor.matmul(den[:, plv:plv + pw], ones[:, 0:1],
                                         pe[:, 0:pw],
                                         start=(pi == 0), stop=(pi == nkb - 1))

                for i, kb in enumerate(kbs):
                    # live q-suffix of this block: fully-masked leading
                    # columns are never computed (causal wedge).
                    ms = (mixed_slot.get((kb, qt))
                          if block_cls[(kb, qt)] == MIXED else None)
                    lv = ms[2] if (ms is not None and ms[0] == 'n'
                                   and i > 0) else 0
                    w = QTA - lv
                    ps = sp.tile([128, QTA], F32, tag="s")
                    nc.tensor.matmul(ps[:, 0:w],
                                     knT[h][:, kb * 128:(kb + 1) * 128],
                                     qnT[h][:, q0 + lv:q0 + QTA],
                                     start=True, stop=False)
                    b0 = (h % 2) * 64
                    nc.tensor.matmul(ps[:, 0:w],
                                     krT[b0:b0 + 64,
                                         kb * 128:(kb + 1) * 128],
                                     _qr(h)[:, q0 + lv:q0 + QTA],
                                     start=False, stop=True)
                    if i == min(4, nkb - 1) and fin_prev is not None:
                        _finalize(*fin_prev)
                        fin_prev = None
                    e = as_.tile([128, QTA], BF16, tag="e", bufs=4)
                    nc.scalar.activation(e[:, 0:w], ps[:, 0:w], AF.Exp,
                                         scale=SCALE)
                    if ms is not None:
                        if ms[0] == 'n':
                            _, sl, _, w0 = ms
                            m0 = nw * QTA + sl * 128
                            co = w0 - lv
                            nc.vector.tensor_mul(
                                e[:, co:co + 128], e[:, co:co + 128],
                                msk_sb[:, m0:m0 + 128])
                        else:
                            sl = ms[1]
                            nc.vector.tensor_mul(
                                e[:], e[:],
                                msk_sb[:, sl * QTA:(sl + 1) * QTA])
                    pend.append((kb, e, i, lv, w))
                    if len(pend) > 3:
                        _flush()
                while pend:
                    _flush()
                rd = as_.tile([1, QTA], F32R, tag="rd", bufs=2)
                with nc.allow_low_precision(reason="f32r broadcast operand"):
                    nc.vector.reciprocal(rd[:], den[:])
                fin_prev = (acc, rd, h)
            _finalize(*fin_prev)
            if level >= 6 and qt < NQA - 1:
                _outproj(op_, os_, qt * QTA, on_dve='all')
        if level < 6:
            _noop_out()
    att_pools.close()
    if level >= 6:
        # final half's projection runs with the attention pools freed so
        # the next rep's KV matmuls can claim PSUM banks immediately.
        _outproj(op_, os_, (NQA - 1) * QTA, on_dve=True)


def _fix_multiwait(nc):
    """This container's walrus only supports ONE sem-wait per instruction.
    Hoist excess waits onto freshly inserted same-engine Drain instructions
    placed immediately before the owner (engine executes in order, so the
    AND-semantics of multiple waits is preserved)."""
    import bass_rust
    n = [0]
    for fn in nc.m.functions:
        for blk in fn.blocks:
            out, changed = [], False
            for inst in blk.instructions:
                si = inst.sync_info
                waits = list(si.on_wait) if (si is not None and si.on_wait) else []
                if len(waits) > 1:
                    changed = True
                    for w in waits[:-1]:
                        n[0] += 1
                        d = bass_rust.InstDrain(
                            name=f"MWFIX-{n[0]}", engine=inst.engine,
                            ins=[], outs=[])
                        d.sync_info = bass_rust.SyncInfo(on_wait=[w],
                                                         on_update=[])
                        out.append(d)
                    si.on_wait = [waits[-1]]
                    inst.sync_info = si
                out.append(inst)
            if changed:
                blk.instructions = out


# ======================= host-side preparation =======================

def _bf16(a):
    return np.asarray(a, np.float32).astype(ml_dtypes.bfloat16)


def rope_tables():
    inv_freq = 1.0 / THETA ** (np.arange(0, DR, 2, dtype=np.float32) / DR)
    pos = np.arange(S, dtype=np.float32)
    freqs = np.outer(pos, inv_freq)
    emb = np.concatenate([freqs, freqs], axis=-1)          # [S, 64]
    cos = np.cos(emb).astype(np.float32)
    sin = np.sin(emb).astype(np.float32)
    sin_s = sin.copy()
    sin_s[:, 0::2] *= -1.0
    return cos, sin_s


def perm_matrix():
    """perm[p, i] = 1 iff p == pairswap(i); symmetric. Block diag x2 for
    the packed 2-head qr tiles."""
    p64 = np.zeros((64, 64), np.float32)
    for i in range(0, 64, 2):
        p64[i + 1, i] = 1.0
        p64[i, i + 1] = 1.0
    out = np.zeros((128, 128), np.float32)
    out[:64, :64] = p64
    out[64:, 64:] = p64
    return out


def analyze_mask(mask):
    """mask: [1,1,S,S] additive. Returns block_cls + packed mask tiles.
    Mixed blocks store ('n', slot, live0, win0) when their fully-masked
    columns form a prefix [0, live0) and the partially-masked columns fit
    one 128-wide window at win0; others store ('w', slot) with the full
    [128, QTA] pattern. Identical patterns dedupe to one slot."""
    global _MASK_SLOTS, _MASK_NS, _MASK_NW
    m = np.asarray(mask, np.float32).reshape(S, S)          # [q, k]
    block_cls = {}
    slot_map = {}
    nar_of, nar = {}, []
    wid_of, wid = {}, []
    for qt in range(NQA):
        first_live = None
        for kb in range(NT):
            sub = m[qt * QTA:(qt + 1) * QTA, kb * 128:(kb + 1) * 128]  # [q,k]
            if np.all(sub <= -1e8):
                block_cls[(kb, qt)] = SKIP
                continue
            if first_live is None:
                first_live = kb
            if np.all(sub == 0.0):
                block_cls[(kb, qt)] = FREE
                continue
            block_cls[(kb, qt)] = MIXED
            t = (sub.T > -1e8).astype(np.float32)       # [k=128, q=QTA]
            anyok = np.any(t == 1.0, axis=0)
            allok = np.all(t == 1.0, axis=0)
            live0 = int(np.argmax(anyok))               # first col w/ any 1
            if kb == first_live:
                live0 = 0                               # start=True coverage
            prefix_dead = not np.any(anyok[:live0])
            bad = np.where(~allok)[0]
            bad = bad[bad >= live0]
            narrow = (prefix_dead and len(bad) > 0
                      and int(bad.max()) - int(bad.min()) < 128)
            if narrow:
                w0 = min(int(bad.min()), QTA - 128)
                pat = t[:, w0:w0 + 128]
                key = pat.tobytes()
                if key not in nar_of:
                    nar_of[key] = len(nar)
                    nar.append(pat)
                slot_map[(kb, qt)] = ('n', nar_of[key], live0, w0)
            else:
                key = t.tobytes()
                if key not in wid_of:
                    wid_of[key] = len(wid)
                    wid.append(t)
                slot_map[(kb, qt)] = ('w', wid_of[key])
    ns, nw = len(nar), len(wid)
    packed = np.zeros((128, max(nw * QTA + ns * 128, 128)), np.float32)
    for sl, t in enumerate(wid):
        packed[:, sl * QTA:(sl + 1) * QTA] = t
    for sl, pat in enumerate(nar):
        c = nw * QTA + sl * 128
        packed[:, c:c + 128] = pat
    _MASK_SLOTS, _MASK_NS, _MASK_NW = slot_map, ns, nw
    return block_cls, _bf16(packed)


def prep_core_inputs(inputs, wqa_cols=WQA_SL):
    """Returns (in_maps list of 8 dicts, block_cls)."""
    x = np.asarray(inputs["x"], np.float32)
    Wqa = np.asarray(inputs["Wqa"], np.float32)
    qw = np.asarray(inputs["q_a_norm_w"], np.float32)
    Wqb = np.asarray(inputs["Wqb"], np.float32)
    Wkva = np.asarray(inputs["Wkva"], np.float32)
    kvw = np.asarray(inputs["kv_a_norm_w"], np.float32)
    Wkvb = np.asarray(inputs["Wkvb"], np.float32)
    Wout = np.asarray(inputs["Wout"], np.float32)

    block_cls, packed = analyze_mask(inputs["attention_mask"])

    wq_eff = Wqa @ (qw[:, None] * Wqb)                      # [D, H*192]
    wq_eff = wq_eff.reshape(D, H, DN + DR)
    wkvb_w = kvw[:, None] * Wkvb                            # [RKV, H*256]
    wkvb_w = wkvb_w.reshape(RKV, H, DN + DV)
    wout_h = Wout.reshape(H, DV, D)

    cos, sin_s = rope_tables()
    cosq = _bf16(np.vstack([cos.T, cos.T]))                 # [128, S]
    sinq = _bf16(np.vstack([sin_s.T, sin_s.T]))
    permq = _bf16(perm_matrix())

    in_maps = []
    for c in range(NCORE):
        b, g = c // TP, c % TP
        hs = slice(g * HPG, (g + 1) * HPG)
        xt_b = _bf16(x[b].T.copy())
        t0 = g * (S // TP)
        m = {
            "xT": xt_b,
            "xkv": np.ascontiguousarray(xt_b[:, t0:t0 + S // TP]),
            "coskv": np.ascontiguousarray(cosq[:, t0:t0 + S // TP]),
            "sinkv": np.ascontiguousarray(sinq[:, t0:t0 + S // TP]),
            "wqa": _bf16(Wqa[:, g * wqa_cols:(g + 1) * wqa_cols]
                         if wqa_cols < RQ else Wqa),
            "wqn": _bf16(wq_eff[:, hs, :DN].reshape(D, HPG * DN)),
            "wqr": _bf16(wq_eff[:, hs, DN:].reshape(D, HPG * DR)),
            "wkva": _bf16(np.concatenate([Wkva, Wkva[:, RKV:]], axis=1)),
            "wkbk": _bf16(wkvb_w[:, hs, :DN].reshape(RKV, HPG * DN)),
            "wkbv": _bf16(wkvb_w[:, hs, DN:].reshape(RKV, HPG * DV)),
            "wout": _bf16(wout_h[hs].reshape(HPG * DV, D)),
            "cosq": cosq,
            "sinq": sinq,
            "perm": permq,
            "masks": packed,
        }
        in_maps.append(m)
    return in_maps, block_cls


def postprocess(results):
    """results: list of 8 dicts with 'outT' [D, S] bf16 partials."""
    out = np.empty((B, S, D), np.float32)
    for b in range(B):
        acc = results[b * TP]["outT"].astype(np.float32).copy()
        for g in range(1, TP):
            acc += results[b * TP + g]["outT"]
        out[b] = acc.T
    return out


# ======================= kernel entry point =======================

_program_cache = {}


def _mask_key(block_cls, packed):
    h = hashlib.sha256()
    h.update(repr(sorted(block_cls.items())).encode())
    h.update(repr(sorted(_MASK_SLOTS.items())).encode())
    h.update(np.ascontiguousarray(packed).tobytes())
    return h.hexdigest()


def kernel(**inputs):
    """Full-input MLA forward on 8 NeuronCores.

    Sharding: data-parallel over batch (2) x tensor-parallel over heads
    (4 groups of 4); the per-token q-RMS statistic is AllReduce'd inside
    each batch group. Host folds Wqa@Wqb, shards weights by head, casts to
    bf16 and transposes x; device returns per-core transposed partial
    outputs which the host sums per batch group.
    """
    from concourse.bass_utils import run_bass_kernel_spmd

    in_maps, block_cls = prep_core_inputs(inputs)
    n_mixed = sum(1 for v in block_cls.values() if v == MIXED)
    key = _mask_key(block_cls, in_maps[0]["masks"])
    nc = _program_cache.get(key)
    if nc is None:
        nc = build_program(block_cls, n_mixed, use_collective=True)
        _program_cache[key] = nc
    res = run_bass_kernel_spmd(nc, in_maps, core_ids=list(range(NCORE)))
    return postprocess(res.results)
